# revision 52
# baseline (speedup 1.0000x reference)
"""Trainium2 Bass kernel for nn_MultiCrossAttention (PVT-style multi-scale
spatial-reduction cross attention) — v2.

Sharding: data-parallel over batch (B=32 -> 4 per core x 8 cores), weights
replicated.  All inputs are cast to bf16 on the host (tolerance is 2e-2; bf16
keeps us ~5e-3) which halves HBM traffic — the memory roofline.

Per-batch pipeline:
  y_i --(contig band DMA, bf16)--> w-pool tree (DVE adds) -> fused
  h-pool+transpose matmuls (PE, pool matrix Ah) -> poolT [c,256] (chan-major).
  Conv runs TOKEN-major: out[tok, c_out] = poolT-chunk^T @ srwT-chunk (+bias
  via K=1 ones-row matmul).  LN stats are then free-axis reductions
  (tensor_reduce / stt accum_out) giving per-token mean/var COLUMNS;
  rstd = exp(-0.5*ln(var+eps)) on the Act engine (Ln+Exp share one
  activation table with the attention Exp — 2 table loads per batch).
  Normalize = (conv - m)*rstd via per-partition tensor_scalar, transpose
  back to chan-major on the PE, and GELU reads the transpose PSUM directly
  with gamma/beta folded into the Act op's per-partition scale/bias.
  x: PE transpose -> xT -> q matmuls; x4 branch same token-major LN trick.
  kv matmuls -> kT (chan-major) + v_aug (token-major, ones column for the
  softmax denominator).  Scores TRANSPOSED (sT[kv,q]) so the denominator
  falls out of the PV matmul's 65th row; normalization via reciprocal +
  rank-1 ones2 broadcast (two heads per matmul) + fused scalar_tensor_tensor.
  proj matmuls (token-major) -> + bias -> out.
"""

import sys

sys.path.insert(0, "/opt/trn_rl_repo")

import numpy as np
import ml_dtypes

import concourse.bass as bass
import concourse.mybir as mybir
import concourse.tile as tile
from concourse.bass_utils import run_bass_kernel_spmd
from concourse.masks import make_identity

# ---------------------------------------------------------------------------
# Patch: this walrus build only accepts ONE sync-wait on a Drain instruction;
# Tile's tail drain waits on every live semaphore lane.  Split it into a chain
# of single-wait drains.
from concourse.vector_clock import ScopedClock, VectorClock
from concourse.tile_sem_assignment import N_PROCS


def _patched_drain_and_barrier(self, tick_clock, wait_clock):
    # Walrus accepts only ONE sync-wait per Drain; instead of a serial chain
    # of single-wait drains on SP, spread them across all five engine queues
    # so the lane waits resolve in parallel, then barrier.
    nc = self.nc
    gc = tick_clock.global_clock
    nz = [p for p in range(N_PROCS) if gc[p] > 0]
    engines = [nc.sync, nc.scalar, nc.vector, nc.gpsimd, nc.tensor]
    for i, p in enumerate(nz):
        masked = VectorClock([gc[q] if q == p else 0 for q in range(N_PROCS)])
        d = engines[i % len(engines)].drain()
        wait_clock.add_sem_waits(d.ins, ScopedClock({None: masked}))
    if not nz:
        nc.sync.drain()
    nc.all_engine_barrier()
    assert self.sems is not None
    popped = nc._tile_sem_poison_stack.pop()
    assert popped is self._sem_poison
    nc.clear_and_free_semaphores(list(self.sems.allocated().values()))
    nc.all_engine_barrier()


tile.TileContext._drain_and_barrier = _patched_drain_and_barrier


def _split_excess_waits(nc, limit=1):
    """Walrus in this build rejects >2 sync-waits on compute/DMA instructions
    (>1 on Drain).  Move excess waits onto same-engine no-ops inserted just
    before the offending instruction."""
    import bass_rust

    uid = [0]
    for f in nc.m.functions:
        for blk in f.blocks:
            newlist = []
            changed = False
            for ins in blk.instructions:
                si = ins.sync_info
                waits = list(si.on_wait) if si and si.on_wait else []
                tn = type(ins).__name__
                lim = 1 if tn in ("InstDrain", "InstNoOp", "InstTensorTensor") else limit
                if len(waits) > lim:
                    keep = waits[-lim:]
                    for w in waits[:-lim]:
                        nop = bass_rust.InstNoOp(
                            name=f"wsplit-{uid[0]}", ins=[], outs=[]
                        )
                        uid[0] += 1
                        nop.engine = ins.engine
                        nop.sync_info = mybir.SyncInfo(on_wait=[w], on_update=[])
                        newlist.append(nop)
                    ins.sync_info = mybir.SyncInfo(
                        on_wait=keep,
                        on_update=list(si.on_update) if si.on_update else [],
                    )
                    changed = True
                newlist.append(ins)
            if changed:
                blk.instructions = newlist


# ---------------------------------------------------------------------------

F32 = mybir.dt.float32
BF16 = mybir.dt.bfloat16
FP8 = mybir.dt.float8e4
PM = mybir.MatmulPerfMode
AF = mybir.ActivationFunctionType
ALU = mybir.AluOpType

NCORES = 8
B = 32
BPC = B // NCORES  # batches per core
N1 = 256  # query tokens
C1 = 512
NH, HD = 8, 64
SCALE = HD ** -0.5
EPS = 1e-5
C2 = (64, 128, 320)
RATIO = (8, 4, 2)
GRP = (1, 2, 4)  # w-groups packed into partitions (128 = H*G)
NKV = 256  # kv tokens (16x16 pooled grid for every branch)

# xc channel-permutation: kt bins of 128 rows; each branch ptile lands at a
# 64-aligned partition base.  Global xc order: x1 0:64 | x2 64:192 | x3
# 192:512 | x4 512:1024.
# kt0=[x1 | x3c], kt1=x2, kt2=x3a, kt3=x3b, kt4..7=x4
_PERM = np.concatenate(
    [
        np.arange(0, 64),  # x1        -> kt0[0:64]
        np.arange(448, 512),  # x3 pt2  -> kt0[64:128]
        np.arange(64, 192),  # x2       -> kt1
        np.arange(192, 320),  # x3 pt0  -> kt2
        np.arange(320, 448),  # x3 pt1  -> kt3
        np.arange(512, 1024),  # x4     -> kt4..7
    ]
)


def _pool_mats():
    """Ah matrices: [128, G*16] mapping partition (h,g) -> col (g*16+ho),
    with the full 1/r^2 divisor folded in."""
    out = []
    for i in range(3):
        G, r = GRP[i], RATIO[i]
        H = 128 // G
        m = np.zeros((128, G * 16), dtype=np.float32)
        for h in range(H):
            for g in range(G):
                p = h * G + g
                ho = h // r
                m[p, g * 16 + ho] = 1.0 / (r * r)
        out.append(m)
    return out


ABLATE = set()


def build_module(reps=1):
    nc = bass.Bass(trn_type="TRN2")

    # ---- DRAM I/O -------------------------------------------------------
    x_d = nc.dram_tensor("x", [BPC, N1, C1], BF16, kind="ExternalInput")
    y1_d = nc.dram_tensor("y1", [BPC, 128 * 128, 64], FP8, kind="ExternalInput")
    y2_d = nc.dram_tensor("y2", [BPC, 64 * 64, 128], FP8, kind="ExternalInput")
    y3_d = nc.dram_tensor("y3", [BPC, 32 * 32, 320], FP8, kind="ExternalInput")
    # split-fp8 weights: W*16 = hi + lo (hi/lo stacked on a leading dim)
    wq_d = nc.dram_tensor("wq_t", [2, C1, C1], FP8, kind="ExternalInput")
    wkv_d = nc.dram_tensor("wkv_t", [2, 1024, 1024], FP8, kind="ExternalInput")
    proj_d = nc.dram_tensor("proj_t", [C1, C1], BF16, kind="ExternalInput")
    projb_d = nc.dram_tensor("projb", [C1], BF16, kind="ExternalInput")
    srw_d = [
        nc.dram_tensor(
            f"srw{i+1}_t",
            [((C2[i] + 127) // 128) * min(C2[i], 128), C2[i]],
            BF16,
            kind="ExternalInput",
        )
        for i in range(3)
    ]
    srb_d = [
        nc.dram_tensor(f"srb{i+1}", [C2[i]], BF16, kind="ExternalInput")
        for i in range(3)
    ]
    ah_d = [
        nc.dram_tensor(f"ah{i+1}", [128, 2, GRP[i] * 16], FP8, kind="ExternalInput")
        for i in range(3)
    ]
    # gamma / beta packed host-side as [nch*128] padded columns
    CB = [64, 128, 320, 512]  # channels per branch (incl. x4)
    NCH = [1, 1, 3, 4]  # 128-channel chunks per branch
    g_d = [
        nc.dram_tensor(f"g{i+1}", [NCH[i] * 128], F32, kind="ExternalInput")
        for i in range(4)
    ]
    b_d = [
        nc.dram_tensor(f"lb{i+1}", [NCH[i] * 128], F32, kind="ExternalInput")
        for i in range(4)
    ]
    out_d = nc.dram_tensor("out", [BPC, N1, C1], BF16, kind="ExternalOutput")

    NPT = [1, 1, 3, 4]  # partition tiles per branch in xcT
    # (kt, base) of each branch ptile in xcT
    XC_SLOT = {
        0: [(0, 0)],
        1: [(1, 0)],
        2: [(2, 0), (3, 0), (0, 64)],
        3: [(4, 0), (5, 0), (6, 0), (7, 0)],
    }

    with tile.TileContext(nc) as tc:
        with (
            tc.tile_pool(name="wts", bufs=1) as wts,
            tc.tile_pool(name="bands", bufs=9) as bandp,
            tc.tile_pool(name="t1", bufs=2) as t1p,
            tc.tile_pool(name="poolt", bufs=2) as pooltp,
            tc.tile_pool(name="work", bufs=2) as work,
            tc.tile_pool(name="xn", bufs=2) as xnp,
            tc.tile_pool(name="scrap", bufs=4) as scrapp,
            tc.tile_pool(name="cols", bufs=2) as colsp,
            tc.tile_pool(name="rows", bufs=2) as rowsp,
            tc.tile_pool(name="ste", bufs=2) as step,
            tc.tile_pool(name="pp", bufs=1, space="PSUM") as pp,
        ):
            y1r = y1_d.ap().rearrange("b (h w) c -> b h (w c)", h=128)
            y2r = y2_d.ap().rearrange("b (h wb wi) c -> b (h wb) (wi c)", wb=2, wi=32)
            y3r = y3_d.ap().rearrange("b (h wb wi) c -> b (h wb) (wi c)", wb=4, wi=8)
            xr = x_d.ap().rearrange("b (nt p) c -> b p nt c", p=128)
            outr = out_d.ap().rearrange("b (nt p) c -> b p nt c", p=128)

            def load_x(bi):
                x_sb = work.tile([128, 2, C1], BF16, tag="x_sb")
                nc.sync.dma_start(out=x_sb, in_=xr[bi])
                return x_sb

            def load_band1(bi, qt):
                band = bandp.tile([128, 2048], FP8, tag="band", name="band1")
                nc.sync.dma_start(
                    out=band, in_=y1r[bi, :, qt * 2048 : (qt + 1) * 2048]
                )
                return band.rearrange("p (wo dw c) -> p wo dw c", wo=4, dw=8)

            def load_band2(bi, hf):
                band = bandp.tile([128, 2048], FP8, tag="band", name="band2")
                nc.sync.dma_start(
                    out=band, in_=y2r[bi, :, hf * 2048 : (hf + 1) * 2048]
                )
                return band.rearrange("p (wo dw c) -> p wo dw c", wo=4, dw=4)

            def load_band3(bi, hf):
                band = bandp.tile([128, 2048], FP8, tag="band", name="band3")
                nc.sync.dma_start(
                    out=band[:, 0:1280],
                    in_=y3r[bi, :, hf * 1280 : (hf + 1) * 1280],
                )
                return band[:, 0:1280].rearrange(
                    "p (wo dw c) -> p wo dw c", wo=2, dw=2
                )

            # ---- batch-0 input DMAs lead the queue (PE's first dependencies)
            pre0 = {"x_sb": load_x(0)}
            pre0["vb1"] = [load_band1(0, qt) for qt in range(4)]
            pre0["vb2"] = [load_band2(0, hf) for hf in range(2)]
            pre0["vb3"] = [load_band3(0, hf) for hf in range(2)]

            # ---- non-DMA constants
            ident = wts.tile([128, 128], BF16)
            make_identity(nc, ident)
            onesrow = wts.tile([1, 128], BF16)
            nc.vector.memset(onesrow, 1.0)
            epscol = wts.tile([128, 1], F32)
            nc.gpsimd.memset(epscol, EPS)

            # ---- weights in first-use order: ah (pool) -> wq (q) -> conv/LN
            ah_s = []
            for i in range(3):
                t = wts.tile([128, 2, GRP[i] * 16], FP8, tag=f"ah{i}", name=f"ah{i}")
                nc.scalar.dma_start(out=t, in_=ah_d[i].ap())
                ah_s.append(t)
            wq_s = wts.tile([128, 2, 4, C1], FP8)
            nc.scalar.dma_start(
                out=wq_s, in_=wq_d.ap().rearrange("s (t p) o -> p s t o", p=128)
            )
            srw_s = []
            for i in range(3):
                c = C2[i]
                nkt = (c + 127) // 128
                t = wts.tile([min(c, 128), nkt, c], BF16, tag=f"srw{i}", name=f"srw{i}")
                nc.scalar.dma_start(
                    out=t, in_=srw_d[i].ap().rearrange("(t p) o -> p t o", p=min(c, 128))
                )
                srw_s.append(t)
            srb_s = [
                wts.tile([1, C2[i]], BF16, tag=f"srb{i}", name=f"srb{i}")
                for i in range(3)
            ]
            for i in range(3):
                nc.scalar.dma_start(
                    out=srb_s[i],
                    in_=bass.AP(tensor=srb_d[i], offset=0, ap=[[0, 1], [1, C2[i]]]),
                )
            g_s, b_s = [], []
            for i in range(4):
                gt = wts.tile([128, NCH[i]], F32, tag=f"g{i}", name=f"g{i}")
                bt = wts.tile([128, NCH[i]], F32, tag=f"b{i}", name=f"b{i}")
                nc.scalar.dma_start(
                    out=gt,
                    in_=bass.AP(tensor=g_d[i], offset=0, ap=[[1, 128], [128, NCH[i]]]),
                )
                nc.scalar.dma_start(
                    out=bt,
                    in_=bass.AP(tensor=b_d[i], offset=0, ap=[[1, 128], [128, NCH[i]]]),
                )
                g_s.append(gt)
                b_s.append(bt)
            projb_s = wts.tile([128, C1], BF16)
            nc.scalar.dma_start(
                out=projb_s,
                in_=bass.AP(tensor=projb_d, offset=0, ap=[[0, 128], [1, C1]]),
            )

            wkv_s = wts.tile([128, 2, 8, 1024], FP8)
            proj_s = wts.tile([128, 4, C1], BF16)

            def bigw_gen():
                # wkv/proj streamed in chunks through window 0's round-robin
                # so batch-0/1 band DMAs interleave rather than queue behind
                # 2.5 MB of weights.  K-half in kp's consumption order (hi
                # pairs (4,5)/(6,7) first), then V-half, proj last.
                wkvr = wkv_d.ap().rearrange("s (t p) o -> p s t o", p=128)
                for s in range(2):
                    nc.scalar.dma_start(
                        out=wkv_s[:, s, 4:8, 0:512], in_=wkvr[:, s, 4:8, 0:512]
                    )
                    yield
                for s in range(2):
                    nc.scalar.dma_start(
                        out=wkv_s[:, s, 0:4, 0:512], in_=wkvr[:, s, 0:4, 0:512]
                    )
                    yield
                for s in range(2):
                    nc.scalar.dma_start(
                        out=wkv_s[:, s, :, 512:1024], in_=wkvr[:, s, :, 512:1024]
                    )
                    yield
                nc.scalar.dma_start(
                    out=proj_s, in_=proj_d.ap().rearrange("(t p) o -> p t o", p=128)
                )

            def s1_gen(bi, st, pre=None):
                """Loads + PE pooling + conv + bn-stats LN + normalize."""
                x_sb = pre["x_sb"] if pre else load_x(bi)
                # x4 stats via bn_stats (free-axis mean/var per token)
                bst4 = colsp.tile([128, 2, 6], F32, tag="bst4", name="bst4")
                mv4 = colsp.tile([128, 2, 2], F32, tag="mv4", name="mv4")
                for nt in range(2):
                    nc.vector.bn_stats(bst4[:, nt], x_sb[:, nt])
                    nc.vector.bn_aggr(mv4[:, nt], bst4[:, nt])
                yield

                # ---- band loads (fp8) ----
                if pre:
                    vb1, vb2, vb3 = pre["vb1"], pre["vb2"], pre["vb3"]
                    yield
                else:
                    vb1 = []
                    for qt in range(4):
                        vb1.append(load_band1(bi, qt))
                        yield
                    vb2 = [load_band2(bi, hf) for hf in range(2)]
                    vb3 = [load_band3(bi, hf) for hf in range(2)]
                    yield

                # x4 cols + xn4
                ln4c = colsp.tile([128, 2], F32, tag="ln4c", name="ln4c")
                nc.scalar.activation(out=ln4c, in_=mv4[:, :, 1], func=AF.Ln, bias=epscol)
                rst4 = colsp.tile([128, 2], F32, tag="rst4", name="rst4")
                nc.scalar.activation(out=rst4, in_=ln4c, func=AF.Exp, scale=-0.5)
                xn4 = work.tile([128, 2, C1], BF16, tag="xn4", bufs=3)
                for nt in range(2):
                    nc.gpsimd.tensor_scalar(
                        xn4[:, nt], x_sb[:, nt],
                        mv4[:, nt, 0:1], rst4[:, nt : nt + 1],
                        ALU.subtract, ALU.mult,
                    )
                st["xn4"] = xn4
                yield

                # x transposes -> xT (fp8); q matmuls (split-fp8 DoubleRow) -> qT
                xT = work.tile([128, 4, NKV], FP8, tag="xT")
                for cp in range(2):
                    tp = pp.tile([128, 2, 2, 128], BF16, tag="ppC", name="xtp", bufs=2)
                    for cl in range(2):
                        ck = cp * 2 + cl
                        for nt in range(2):
                            nc.tensor.transpose(
                                tp[:, cl, nt],
                                x_sb[:, nt, ck * 128 : (ck + 1) * 128],
                                ident,
                            )
                    nc.vector.tensor_copy(
                        xT[:, cp * 2 : (cp + 1) * 2],
                        tp.rearrange("p a b c -> p (a b c)"),
                    )
                    yield
                qT = work.tile([128, 4, NKV], BF16, tag="qT", bufs=4)
                for mp in range(2):
                    qp = pp.tile([128, 2, NKV], F32, tag="ppB", name="qp", bufs=2)
                    for ml in range(2):
                        mt = mp * 2 + ml
                        idx = 0
                        for kpr in range(2):
                            for s in range(2):
                                nc.tensor.matmul(
                                    qp[:, ml],
                                    wq_s[:, s, 2 * kpr : 2 * kpr + 2,
                                         mt * 128 : (mt + 1) * 128],
                                    xT[:, 2 * kpr : 2 * kpr + 2],
                                    start=(idx == 0),
                                    stop=(idx == 3),
                                    perf_mode=PM.DoubleRow,
                                    skip_group_check=True,
                                )
                                idx += 1
                    nc.vector.tensor_copy(
                        qT[:, mp * 2 : (mp + 1) * 2],
                        qp,
                    )
                    yield
                st["qT"] = qT

                # ---- fused w+h-pool on the PE (fp8 DoubleRow over dw pairs)
                poolp1 = pp.tile([64, 16, 16], F32, tag="ppA", name="poolp1", bufs=2)
                for qt in range(4):
                    for wl in range(4):
                        for dp in range(4):
                            nc.tensor.matmul(
                                poolp1[:, qt * 4 + wl],
                                vb1[qt][:, wl, 2 * dp : 2 * dp + 2],
                                ah_s[0],
                                start=(dp == 0),
                                stop=(dp == 3),
                                perf_mode=PM.DoubleRow,
                                skip_group_check=True,
                            )
                poolt1 = pooltp.tile([64, NKV], BF16, tag="poolt1")
                nc.scalar.copy(out=poolt1, in_=poolp1.rearrange("c a b -> c (a b)"))
                yield

                poolp2 = pp.tile([128, 2, 8, 16], F32, tag="ppA", name="poolp2", bufs=2)
                for hf in range(2):
                    for wl in range(4):
                        for dp in range(2):
                            nc.tensor.matmul(
                                poolp2[:, :, hf * 4 + wl],
                                vb2[hf][:, wl, 2 * dp : 2 * dp + 2],
                                ah_s[1],
                                start=(dp == 0),
                                stop=(dp == 1),
                                perf_mode=PM.DoubleRow,
                                skip_group_check=True,
                            )
                poolt2 = pooltp.tile([128, NKV], BF16, tag="poolt2")
                nc.scalar.copy(out=poolt2, in_=poolp2.rearrange("c g a b -> c (g a b)"))
                yield

                poolt3 = pooltp.tile([128, 3, NKV], BF16, tag="poolt3")
                for cs in range(3):
                    cl = 64 if cs == 2 else 128
                    poolp3 = pp.tile([128, 4, 4, 16], F32, tag="ppA", name="poolp3", bufs=2)
                    for hf in range(2):
                        for wl in range(2):
                            nc.tensor.matmul(
                                poolp3[:cl, :, hf * 2 + wl],
                                vb3[hf][:, wl, :, cs * 128 : cs * 128 + cl],
                                ah_s[2],
                                start=True,
                                stop=True,
                                perf_mode=PM.DoubleRow,
                                skip_group_check=True,
                            )
                    nc.scalar.copy(
                        out=poolt3[:cl, cs],
                        in_=poolp3[:cl].rearrange("c g a b -> c (g a b)"),
                    )
                    yield

                # ---- branch conv (token-major) + bn-stats LN + normalize
                poolts = [poolt1, poolt2, poolt3]
                xns = []
                if "conv" in ABLATE:
                    for br in range(3):
                        xn = xnp.tile([128, 2, C2[br]], BF16, tag=f"xn{br}", name=f"xn{br}")
                        nc.vector.memset(xn, 0.2)
                        xns.append(xn)
                    st["xns"] = xns
                    yield
                    return
                xns = [None, None, None]
                for br in [2, 1, 0]:
                    cb = C2[br]
                    nkt = (cb + 127) // 128
                    xn = xnp.tile([128, 2, cb], BF16, tag=f"xn{br}", name=f"xn{br}")
                    bst = colsp.tile([128, 2, 6], F32, tag=f"bst{br}", name=f"bst{br}")
                    mv = colsp.tile([128, 2, 2], F32, tag=f"mv{br}", name=f"mv{br}")
                    lnc = colsp.tile([128, 2], F32, tag=f"ln{br}", name=f"ln{br}")
                    rstd = colsp.tile([128, 2], F32, tag=f"rst{br}", name=f"rst{br}")
                    preps = []
                    for tc in range(2):
                        prep = pp.tile([128, 320], F32, tag="ppA", name=f"prep{br}", bufs=2)
                        nc.tensor.matmul(
                            prep[:, 0:cb],
                            onesrow,
                            srb_s[br],
                            start=True,
                            stop=False,
                        )
                        for kt in range(nkt):
                            kl = min(128, cb - kt * 128)
                            if br < 2:
                                lhs = poolts[br][:kl, tc * 128 : (tc + 1) * 128]
                            else:
                                lhs = poolts[2][:kl, kt, tc * 128 : (tc + 1) * 128]
                            nc.tensor.matmul(
                                prep[:, 0:cb],
                                lhs,
                                srw_s[br][:kl, kt],
                                start=False,
                                stop=(kt == nkt - 1),
                            )
                        nc.vector.bn_stats(bst[:, tc], prep[:, 0:cb])
                        nc.vector.bn_aggr(mv[:, tc], bst[:, tc])
                        preps.append(prep)
                        if tc == 0:
                            yield
                    # one Ln/Exp pair per branch (both tc halves at once)
                    nc.scalar.activation(
                        out=lnc, in_=mv[:, :, 1], func=AF.Ln, bias=epscol
                    )
                    nc.scalar.activation(out=rstd, in_=lnc, func=AF.Exp, scale=-0.5)
                    for tc in range(2):
                        nc.vector.tensor_scalar(
                            xn[:, tc], preps[tc][:, 0:cb],
                            mv[:, tc, 0:1], rstd[:, tc : tc + 1],
                            ALU.subtract, ALU.mult,
                        )
                    yield
                    xns[br] = xn
                st["xns"] = xns

            def s2ab_gen(bi, st):
                """Back-transposes + GELU -> xcT, then kv matmuls
                (split-fp8 DoubleRow) — chunked into the window round-robin."""
                xn4, xns = st["xn4"], st["xns"]
                xcT = work.tile([128, 8, NKV], FP8, tag="xcT")
                for ck in range(4):
                    tp4 = pp.tile([128, 2, 128], BF16, tag="ppC", name="tp4", bufs=2)
                    for nt in range(2):
                        nc.tensor.transpose(
                            tp4[:, nt], xn4[:, nt, ck * 128 : (ck + 1) * 128], ident
                        )
                    dst = xcT[:, 4 + ck]
                    nc.scalar.activation(
                        out=dst.rearrange("c (wo ho) -> c ho wo", wo=16),
                        in_=tp4.rearrange("c nt (hh wo) -> c (nt hh) wo", hh=8),
                        func=AF.Gelu,
                        scale=g_s[3][:, ck : ck + 1],
                        bias=b_s[3][:, ck : ck + 1],
                    )
                    if ck % 2 == 1:
                        yield

                for br in [1, 2, 0]:
                    cb = C2[br]
                    xn = xns[br]
                    for ch in range(NPT[br]):
                        cl = min(128, cb - ch * 128)
                        kt_slot, base = XC_SLOT[br][ch]
                        tpb = pp.tile([128, 2, 128], BF16, tag="ppC", name=f"tpb{br}", bufs=2)
                        for tc in range(2):
                            nc.tensor.transpose(
                                tpb[:cl, tc],
                                xn[:, tc, ch * 128 : ch * 128 + cl],
                                ident,
                            )
                        nc.scalar.activation(
                            out=xcT[base : base + cl, kt_slot],
                            in_=tpb[:cl].rearrange("c a b -> c (a b)"),
                            func=AF.Gelu,
                            scale=g_s[br][0:cl, ch : ch + 1],
                            bias=b_s[br][0:cl, ch : ch + 1],
                        )
                    yield
                st["xcT"] = xcT

                # ---- kv matmuls ----
                # kt pairs in xcT-readiness order: x4 (4..7), then branches
                PAIRS = [4, 6, 2, 0]
                kT = work.tile([128, 4, NKV], BF16, tag="kT")
                if "kv" in ABLATE:
                    nc.vector.memset(kT, 0.1)
                    st["kT"] = kT
                    v_aug = work.tile([128, 2, NH, HD + 1], BF16, tag="v_aug")
                    nc.vector.memset(v_aug, 0.1)
                    st["v_aug"] = v_aug
                    yield
                    return
                for mp in range(2):
                    kp = pp.tile([128, 2, NKV], F32, tag="ppB", name="kp", bufs=2)
                    for ml in range(2):
                        mt = mp * 2 + ml
                        idx = 0
                        for a in PAIRS:
                            for s in range(2):
                                nc.tensor.matmul(
                                    kp[:, ml],
                                    wkv_s[:, s, a : a + 2, mt * 128 : (mt + 1) * 128],
                                    xcT[:, a : a + 2],
                                    start=(idx == 0),
                                    stop=(idx == 7),
                                    perf_mode=PM.DoubleRow,
                                    skip_group_check=True,
                                )
                                idx += 1
                    nc.scalar.copy(out=kT[:, mp * 2 : (mp + 1) * 2], in_=kp)
                    yield
                st["kT"] = kT

                v_aug = work.tile([128, 2, NH, HD + 1], FP8, tag="v_aug")
                nc.vector.memset(v_aug[:, :, :, HD : HD + 1], 1.0)
                for mt in range(2):
                    vp = pp.tile([128, 2, NKV], F32, tag="ppB", name="vp", bufs=2)
                    for vh in range(2):
                        idx = 0
                        for a in PAIRS:
                            for s in range(2):
                                nc.tensor.matmul(
                                    vp[:, vh],
                                    xcT[:, a : a + 2, mt * 128 : (mt + 1) * 128],
                                    wkv_s[:, s, a : a + 2,
                                          512 + vh * 256 : 768 + vh * 256],
                                    start=(idx == 0),
                                    stop=(idx == 7),
                                    perf_mode=PM.DoubleRow,
                                    skip_group_check=True,
                                )
                                idx += 1
                    nc.scalar.copy(
                        out=v_aug[:, mt, :, 0:HD],
                        in_=vp.rearrange("p a (h d) -> p (a h) d", h=4),
                    )
                    yield
                st["v_aug"] = v_aug

            def s3_gen(bi, st):
                """Attention (head-pipelined) + proj + store."""
                qT, kT, v_aug = st["qT"], st["kT"], st["v_aug"]
                outT = work.tile([128, 4, NKV], BF16, tag="outT")
                if "attn" in ABLATE:
                    nc.vector.memset(outT, 0.5)
                    yield
                else:
                    sps, stes, pv2s, rss = {}, {}, {}, {}

                    def emit_sp(h):
                        j, hh = h // 2, h % 2
                        pb = hh * 64
                        sp = pp.tile([128, 2, NKV], F32, tag="ppC", name="sp", bufs=2)
                        for nt in range(2):
                            nc.tensor.matmul(
                                sp[:, nt],
                                kT[pb : pb + 64, j, nt * 128 : (nt + 1) * 128],
                                qT[pb : pb + 64, j],
                                start=True,
                                stop=True,
                                skip_group_check=True,
                            )
                        ste = step.tile([128, 2, NKV], FP8, tag="ste")
                        nc.scalar.activation(
                            out=ste, in_=sp, func=AF.Exp, scale=SCALE / 256.0
                        )
                        stes[h] = ste

                    def emit_pv(h):
                        j, hh = h // 2, h % 2
                        if hh == 0:
                            pv2s[j] = pp.tile([65, 2, NKV], F32, tag="ppD", name="pv2", bufs=2)
                        nc.tensor.matmul(
                            pv2s[j][:, hh],
                            v_aug[:, :, h],
                            stes[h],
                            start=True,
                            stop=True,
                            perf_mode=PM.DoubleRow,
                            skip_group_check=True,
                        )
                        del stes[h]

                    def emit_norm(j):
                        pv2 = pv2s[j]
                        rs2 = rowsp.tile([1, 2, NKV], BF16, tag="rs2")
                        bc = pp.tile([128, NKV], F32, tag="ppB", name="bc", bufs=2)
                        with nc.allow_low_precision(reason="bf16 softmax denom"):
                            nc.vector.reciprocal(rs2, pv2[64:65])
                        nc.tensor.matmul(
                            bc[0:64], onesrow[:, 0:64], rs2[:, 0],
                            start=True, stop=True, skip_group_check=True,
                        )
                        nc.tensor.matmul(
                            bc[64:128], onesrow[:, 0:64], rs2[:, 1],
                            start=True, stop=True, skip_group_check=True,
                        )
                        for hh in range(2):
                            pb = hh * 64
                            nc.vector.scalar_tensor_tensor(
                                out=outT[pb : pb + 64, j],
                                in0=pv2[0:64, hh], scalar=1.0, in1=bc[pb : pb + 64],
                                op0=ALU.mult, op1=ALU.mult,
                            )
                        del pv2s[j]

                    # head-level software pipeline: sp(h+1) issued between
                    # exp(h) and pv(h); pair tails interleave two heads later
                    emit_sp(0)
                    for h in range(NH):
                        if h + 1 < NH:
                            emit_sp(h + 1)
                        emit_pv(h)
                        if h >= 2 and h % 2 == 1:
                            emit_norm(h // 2 - 1)
                            yield
                    emit_norm(3)
                    yield

                osb = work.tile([128, 2, C1], BF16, tag="osb")
                for tc in range(2):
                    fp = pp.tile([128, 2, NKV], F32, tag="ppB", name="fp", bufs=2)
                    kts = [0, 1, 2, 3]
                    for fh in range(2):
                        for kt in kts:
                            nc.tensor.matmul(
                                fp[:, fh],
                                outT[:, kt, tc * 128 : (tc + 1) * 128],
                                proj_s[:, kt, fh * 256 : (fh + 1) * 256],
                                start=(kt == 0),
                                stop=(kt == 3),
                                skip_group_check=True,
                            )
                    # alternate engines so the two bias-adds overlap, and
                    # store each half as soon as it is ready
                    nc.vector.tensor_add(
                        osb[:, tc],
                        fp.rearrange("p a b -> p (a b)"),
                        projb_s,
                    )
                    nc.sync.dma_start(out=outr[bi][:, tc], in_=osb[:, tc])
                    yield

            def _drain(g):
                if g is None:
                    return False
                try:
                    next(g)
                    return True
                except StopIteration:
                    return False

            # ---- software pipeline ------------------------------------
            # Window t round-robins chunks of S3(t-2) / S2ab(t-1) / S1y(t) /
            # S1x(t+1); window 0 also streams the big weights between band
            # DMAs and runs S1x(0).
            NB = reps * BPC
            states = {}
            for t in range(NB + 2):
                gens = []
                if t < NB:
                    states[t] = {}
                if t >= 2:
                    gens.append(s3_gen((t - 2) % BPC, states[t - 2]))
                if 1 <= t and t - 1 < NB:
                    gens.append(s2ab_gen((t - 1) % BPC, states[t - 1]))
                if t < NB:
                    gens.append(s1_gen(t % BPC, states[t],
                                       pre=pre0 if t == 0 else None))
                if t == 0:
                    gens.append(bigw_gen())
                while gens:
                    nxt = []
                    for g in gens:
                        try:
                            next(g)
                            nxt.append(g)
                        except StopIteration:
                            pass
                    gens = nxt
                if t >= 2:
                    del states[t - 2]

    _split_excess_waits(nc)
    return nc


def _split_fp8(w16):
    """w16 (f32) -> stacked [2, ...] fp8 hi/lo with hi+lo ~= w16."""
    f8 = ml_dtypes.float8_e4m3
    hi = w16.astype(f8)
    lo = (w16 - hi.astype(np.float32)).astype(f8)
    return np.stack([hi, lo], axis=0)


def _prep_common(inputs):
    Wq = np.asarray(inputs["Wq"], dtype=np.float32)
    Wkv = np.asarray(inputs["Wkv"], dtype=np.float32)
    proj_w = np.asarray(inputs["proj_w"], dtype=np.float32)
    proj_b = np.asarray(inputs["proj_b"], dtype=np.float32)

    bf = ml_dtypes.bfloat16
    f8 = ml_dtypes.float8_e4m3
    common = {
        # x16 prescale keeps the fp8 split residual out of subnormals; the
        # 16*16=256 score scale folds into the Exp activation, the 16x on v
        # folds into proj_t below.
        "wq_t": _split_fp8(np.ascontiguousarray(Wq.T) * 16.0),
        "wkv_t": _split_fp8(np.ascontiguousarray(Wkv.T[_PERM, :]) * 16.0),
        "proj_t": (np.ascontiguousarray(proj_w.T) / 16.0).astype(bf),
        "projb": proj_b.astype(bf),
    }
    ah = _pool_mats()
    for i in range(3):
        common[f"ah{i+1}"] = np.stack([ah[i], ah[i]], axis=1).astype(f8)
        c = C2[i]
        cpad = ((c + 127) // 128) * 128
        pr = min(c, 128)
        nkt = (c + 127) // 128
        srw_t = np.asarray(inputs[f"sr{i+1}_w"], dtype=np.float32).T  # [c_in, c_out]
        srw_p = np.zeros((nkt * pr, c), dtype=np.float32)
        srw_p[:c] = srw_t
        common[f"srw{i+1}_t"] = srw_p.astype(bf)
        common[f"srb{i+1}"] = np.asarray(
            inputs[f"sr{i+1}_b"], dtype=np.float32
        ).astype(bf)
    for i, c in enumerate((64, 128, 320, 512)):
        cpad = ((c + 127) // 128) * 128
        if i < 3:
            g = np.asarray(inputs[f"ln{i+1}_g"], dtype=np.float32)
            b = np.asarray(inputs[f"ln{i+1}_b"], dtype=np.float32)
        else:
            g = np.asarray(inputs["ln4_g"], dtype=np.float32)
            b = np.asarray(inputs["ln4_b"], dtype=np.float32)
        gp = np.zeros(cpad, dtype=np.float32)
        gp[:c] = g
        bp = np.zeros(cpad, dtype=np.float32)
        bp[:c] = b
        common[f"g{i+1}"] = gp
        common[f"lb{i+1}"] = bp
    return common


def kernel(**inputs):
    bf = ml_dtypes.bfloat16
    f8 = ml_dtypes.float8_e4m3
    x = np.ascontiguousarray(inputs["x"]).astype(bf)
    y1 = np.ascontiguousarray(inputs["y1"]).astype(f8)
    y2 = np.ascontiguousarray(inputs["y2"]).astype(f8)
    y3 = np.ascontiguousarray(inputs["y3"]).astype(f8)
    common = _prep_common(inputs)

    nc = build_module()
    in_maps = []
    for c in range(NCORES):
        sl = slice(c * BPC, (c + 1) * BPC)
        m = dict(common)
        m["x"] = x[sl]
        m["y1"] = y1[sl]
        m["y2"] = y2[sl]
        m["y3"] = y3[sl]
        in_maps.append(m)

    res = run_bass_kernel_spmd(nc, in_maps, core_ids=list(range(NCORES)))
    return np.concatenate(
        [np.asarray(r["out"]).astype(np.float32) for r in res.results], axis=0
    )


if __name__ == "__main__":
    pass



# revision 53
# speedup vs baseline: 1.0014x; 1.0014x over previous
"""Trainium2 Bass kernel for nn_MultiCrossAttention (PVT-style multi-scale
spatial-reduction cross attention) — v2.

Sharding: data-parallel over batch (B=32 -> 4 per core x 8 cores), weights
replicated.  All inputs are cast to bf16 on the host (tolerance is 2e-2; bf16
keeps us ~5e-3) which halves HBM traffic — the memory roofline.

Per-batch pipeline:
  y_i --(contig band DMA, bf16)--> w-pool tree (DVE adds) -> fused
  h-pool+transpose matmuls (PE, pool matrix Ah) -> poolT [c,256] (chan-major).
  Conv runs TOKEN-major: out[tok, c_out] = poolT-chunk^T @ srwT-chunk (+bias
  via K=1 ones-row matmul).  LN stats are then free-axis reductions
  (tensor_reduce / stt accum_out) giving per-token mean/var COLUMNS;
  rstd = exp(-0.5*ln(var+eps)) on the Act engine (Ln+Exp share one
  activation table with the attention Exp — 2 table loads per batch).
  Normalize = (conv - m)*rstd via per-partition tensor_scalar, transpose
  back to chan-major on the PE, and GELU reads the transpose PSUM directly
  with gamma/beta folded into the Act op's per-partition scale/bias.
  x: PE transpose -> xT -> q matmuls; x4 branch same token-major LN trick.
  kv matmuls -> kT (chan-major) + v_aug (token-major, ones column for the
  softmax denominator).  Scores TRANSPOSED (sT[kv,q]) so the denominator
  falls out of the PV matmul's 65th row; normalization via reciprocal +
  rank-1 ones2 broadcast (two heads per matmul) + fused scalar_tensor_tensor.
  proj matmuls (token-major) -> + bias -> out.
"""

import sys

sys.path.insert(0, "/opt/trn_rl_repo")

import numpy as np
import ml_dtypes

import concourse.bass as bass
import concourse.mybir as mybir
import concourse.tile as tile
from concourse.bass_utils import run_bass_kernel_spmd
from concourse.masks import make_identity

# ---------------------------------------------------------------------------
# Patch: this walrus build only accepts ONE sync-wait on a Drain instruction;
# Tile's tail drain waits on every live semaphore lane.  Split it into a chain
# of single-wait drains.
from concourse.vector_clock import ScopedClock, VectorClock
from concourse.tile_sem_assignment import N_PROCS


def _patched_drain_and_barrier(self, tick_clock, wait_clock):
    # Walrus accepts only ONE sync-wait per Drain; instead of a serial chain
    # of single-wait drains on SP, spread them across all five engine queues
    # so the lane waits resolve in parallel, then barrier.
    nc = self.nc
    gc = tick_clock.global_clock
    nz = [p for p in range(N_PROCS) if gc[p] > 0]
    engines = [nc.sync, nc.scalar, nc.vector, nc.gpsimd, nc.tensor]
    for i, p in enumerate(nz):
        masked = VectorClock([gc[q] if q == p else 0 for q in range(N_PROCS)])
        d = engines[i % len(engines)].drain()
        wait_clock.add_sem_waits(d.ins, ScopedClock({None: masked}))
    if not nz:
        nc.sync.drain()
    nc.all_engine_barrier()
    assert self.sems is not None
    popped = nc._tile_sem_poison_stack.pop()
    assert popped is self._sem_poison
    nc.clear_and_free_semaphores(list(self.sems.allocated().values()))
    nc.all_engine_barrier()


tile.TileContext._drain_and_barrier = _patched_drain_and_barrier


def _split_excess_waits(nc, limit=1):
    """Walrus in this build rejects >2 sync-waits on compute/DMA instructions
    (>1 on Drain).  Move excess waits onto same-engine no-ops inserted just
    before the offending instruction."""
    import bass_rust

    uid = [0]
    for f in nc.m.functions:
        for blk in f.blocks:
            newlist = []
            changed = False
            for ins in blk.instructions:
                si = ins.sync_info
                waits = list(si.on_wait) if si and si.on_wait else []
                tn = type(ins).__name__
                lim = 1 if tn in ("InstDrain", "InstNoOp", "InstTensorTensor") else limit
                if len(waits) > lim:
                    keep = waits[-lim:]
                    for w in waits[:-lim]:
                        nop = bass_rust.InstNoOp(
                            name=f"wsplit-{uid[0]}", ins=[], outs=[]
                        )
                        uid[0] += 1
                        nop.engine = ins.engine
                        nop.sync_info = mybir.SyncInfo(on_wait=[w], on_update=[])
                        newlist.append(nop)
                    ins.sync_info = mybir.SyncInfo(
                        on_wait=keep,
                        on_update=list(si.on_update) if si.on_update else [],
                    )
                    changed = True
                newlist.append(ins)
            if changed:
                blk.instructions = newlist


# ---------------------------------------------------------------------------

F32 = mybir.dt.float32
BF16 = mybir.dt.bfloat16
FP8 = mybir.dt.float8e4
PM = mybir.MatmulPerfMode
AF = mybir.ActivationFunctionType
ALU = mybir.AluOpType

NCORES = 8
B = 32
BPC = B // NCORES  # batches per core
N1 = 256  # query tokens
C1 = 512
NH, HD = 8, 64
SCALE = HD ** -0.5
EPS = 1e-5
C2 = (64, 128, 320)
RATIO = (8, 4, 2)
GRP = (1, 2, 4)  # w-groups packed into partitions (128 = H*G)
NKV = 256  # kv tokens (16x16 pooled grid for every branch)

# xc channel-permutation: kt bins of 128 rows; each branch ptile lands at a
# 64-aligned partition base.  Global xc order: x1 0:64 | x2 64:192 | x3
# 192:512 | x4 512:1024.
# kt0=[x1 | x3c], kt1=x2, kt2=x3a, kt3=x3b, kt4..7=x4
_PERM = np.concatenate(
    [
        np.arange(0, 64),  # x1        -> kt0[0:64]
        np.arange(448, 512),  # x3 pt2  -> kt0[64:128]
        np.arange(64, 192),  # x2       -> kt1
        np.arange(192, 320),  # x3 pt0  -> kt2
        np.arange(320, 448),  # x3 pt1  -> kt3
        np.arange(512, 1024),  # x4     -> kt4..7
    ]
)


def _pool_mats():
    """Ah matrices: [128, G*16] mapping partition (h,g) -> col (g*16+ho),
    with the full 1/r^2 divisor folded in."""
    out = []
    for i in range(3):
        G, r = GRP[i], RATIO[i]
        H = 128 // G
        m = np.zeros((128, G * 16), dtype=np.float32)
        for h in range(H):
            for g in range(G):
                p = h * G + g
                ho = h // r
                m[p, g * 16 + ho] = 1.0 / (r * r)
        out.append(m)
    return out


ABLATE = set()


def build_module(reps=1):
    nc = bass.Bass(trn_type="TRN2")

    # ---- DRAM I/O -------------------------------------------------------
    x_d = nc.dram_tensor("x", [BPC, N1, C1], BF16, kind="ExternalInput")
    y1_d = nc.dram_tensor("y1", [BPC, 128 * 128, 64], FP8, kind="ExternalInput")
    y2_d = nc.dram_tensor("y2", [BPC, 64 * 64, 128], FP8, kind="ExternalInput")
    y3_d = nc.dram_tensor("y3", [BPC, 32 * 32, 320], FP8, kind="ExternalInput")
    # split-fp8 weights: W*16 = hi + lo (hi/lo stacked on a leading dim)
    wq_d = nc.dram_tensor("wq_t", [2, C1, C1], FP8, kind="ExternalInput")
    wkv_d = nc.dram_tensor("wkv_t", [2, 1024, 1024], FP8, kind="ExternalInput")
    proj_d = nc.dram_tensor("proj_t", [C1, C1], BF16, kind="ExternalInput")
    projb_d = nc.dram_tensor("projb", [C1], BF16, kind="ExternalInput")
    srw_d = [
        nc.dram_tensor(
            f"srw{i+1}_t",
            [((C2[i] + 127) // 128) * min(C2[i], 128), C2[i]],
            BF16,
            kind="ExternalInput",
        )
        for i in range(3)
    ]
    srb_d = [
        nc.dram_tensor(f"srb{i+1}", [C2[i]], BF16, kind="ExternalInput")
        for i in range(3)
    ]
    ah_d = [
        nc.dram_tensor(f"ah{i+1}", [128, 2, GRP[i] * 16], FP8, kind="ExternalInput")
        for i in range(3)
    ]
    # gamma / beta packed host-side as [nch*128] padded columns
    CB = [64, 128, 320, 512]  # channels per branch (incl. x4)
    NCH = [1, 1, 3, 4]  # 128-channel chunks per branch
    g_d = [
        nc.dram_tensor(f"g{i+1}", [NCH[i] * 128], F32, kind="ExternalInput")
        for i in range(4)
    ]
    b_d = [
        nc.dram_tensor(f"lb{i+1}", [NCH[i] * 128], F32, kind="ExternalInput")
        for i in range(4)
    ]
    out_d = nc.dram_tensor("out", [BPC, N1, C1], BF16, kind="ExternalOutput")

    NPT = [1, 1, 3, 4]  # partition tiles per branch in xcT
    # (kt, base) of each branch ptile in xcT
    XC_SLOT = {
        0: [(0, 0)],
        1: [(1, 0)],
        2: [(2, 0), (3, 0), (0, 64)],
        3: [(4, 0), (5, 0), (6, 0), (7, 0)],
    }

    with tile.TileContext(nc) as tc:
        with (
            tc.tile_pool(name="wts", bufs=1) as wts,
            tc.tile_pool(name="bands", bufs=2) as bandp,
            tc.tile_pool(name="t1", bufs=2) as t1p,
            tc.tile_pool(name="poolt", bufs=2) as pooltp,
            tc.tile_pool(name="work", bufs=2) as work,
            tc.tile_pool(name="xn", bufs=2) as xnp,
            tc.tile_pool(name="scrap", bufs=4) as scrapp,
            tc.tile_pool(name="cols", bufs=2) as colsp,
            tc.tile_pool(name="rows", bufs=2) as rowsp,
            tc.tile_pool(name="ste", bufs=2) as step,
            tc.tile_pool(name="pp", bufs=1, space="PSUM") as pp,
        ):
            y1r = y1_d.ap().rearrange("b (h w) c -> b h (w c)", h=128)
            y2r = y2_d.ap().rearrange("b (h wb wi) c -> b (h wb) (wi c)", wb=2, wi=32)
            y3r = y3_d.ap().rearrange("b (h wb wi) c -> b (h wb) (wi c)", wb=4, wi=8)
            xr = x_d.ap().rearrange("b (nt p) c -> b p nt c", p=128)
            outr = out_d.ap().rearrange("b (nt p) c -> b p nt c", p=128)

            def load_x(bi):
                x_sb = work.tile([128, 2, C1], BF16, tag="x_sb")
                nc.sync.dma_start(out=x_sb, in_=xr[bi])
                return x_sb

            def load_bands(bi, split_y1=False):
                """One DMA per y tensor (HWDGE dispatch is ~625ns/DMA and a
                co-bottleneck); split_y1 chunks y1 so batch-0 pooling can
                start on the first quarter."""
                b1 = bandp.tile([128, 4, 2048], FP8, tag="band1", name="band1",
                                bufs=2)
                if split_y1:
                    for qt in range(4):
                        nc.sync.dma_start(
                            out=b1[:, qt],
                            in_=y1r[bi, :, qt * 2048 : (qt + 1) * 2048],
                        )
                else:
                    nc.sync.dma_start(
                        out=b1, in_=y1r[bi].rearrange("p (q x) -> p q x", q=4)
                    )
                b2 = bandp.tile([128, 2, 2048], FP8, tag="band2", name="band2",
                                bufs=2)
                nc.sync.dma_start(
                    out=b2, in_=y2r[bi].rearrange("p (q x) -> p q x", q=2)
                )
                b3 = bandp.tile([128, 2, 1280], FP8, tag="band3", name="band3",
                                bufs=2)
                nc.sync.dma_start(
                    out=b3, in_=y3r[bi].rearrange("p (q x) -> p q x", q=2)
                )
                vb1 = [b1[:, qt].rearrange("p (wo dw c) -> p wo dw c", wo=4, dw=8)
                       for qt in range(4)]
                vb2 = [b2[:, hf].rearrange("p (wo dw c) -> p wo dw c", wo=4, dw=4)
                       for hf in range(2)]
                vb3 = [b3[:, hf].rearrange("p (wo dw c) -> p wo dw c", wo=2, dw=2)
                       for hf in range(2)]
                return vb1, vb2, vb3

            # ---- batch-0 input DMAs lead the queue (PE's first dependencies)
            pre0 = {"x_sb": load_x(0)}
            pre0["vb1"], pre0["vb2"], pre0["vb3"] = load_bands(0, split_y1=True)

            # ---- non-DMA constants
            ident = wts.tile([128, 128], BF16)
            make_identity(nc, ident)
            onesrow = wts.tile([1, 128], BF16)
            nc.vector.memset(onesrow, 1.0)
            epscol = wts.tile([128, 1], F32)
            nc.gpsimd.memset(epscol, EPS)

            # ---- weights in first-use order: ah (pool) -> wq (q) -> conv/LN
            ah_s = []
            for i in range(3):
                t = wts.tile([128, 2, GRP[i] * 16], FP8, tag=f"ah{i}", name=f"ah{i}")
                nc.scalar.dma_start(out=t, in_=ah_d[i].ap())
                ah_s.append(t)
            wq_s = wts.tile([128, 2, 4, C1], FP8)
            nc.scalar.dma_start(
                out=wq_s, in_=wq_d.ap().rearrange("s (t p) o -> p s t o", p=128)
            )
            srw_s = []
            for i in range(3):
                c = C2[i]
                nkt = (c + 127) // 128
                t = wts.tile([min(c, 128), nkt, c], BF16, tag=f"srw{i}", name=f"srw{i}")
                nc.scalar.dma_start(
                    out=t, in_=srw_d[i].ap().rearrange("(t p) o -> p t o", p=min(c, 128))
                )
                srw_s.append(t)
            srb_s = [
                wts.tile([1, C2[i]], BF16, tag=f"srb{i}", name=f"srb{i}")
                for i in range(3)
            ]
            for i in range(3):
                nc.scalar.dma_start(
                    out=srb_s[i],
                    in_=bass.AP(tensor=srb_d[i], offset=0, ap=[[0, 1], [1, C2[i]]]),
                )
            g_s, b_s = [], []
            for i in range(4):
                gt = wts.tile([128, NCH[i]], F32, tag=f"g{i}", name=f"g{i}")
                bt = wts.tile([128, NCH[i]], F32, tag=f"b{i}", name=f"b{i}")
                nc.scalar.dma_start(
                    out=gt,
                    in_=bass.AP(tensor=g_d[i], offset=0, ap=[[1, 128], [128, NCH[i]]]),
                )
                nc.scalar.dma_start(
                    out=bt,
                    in_=bass.AP(tensor=b_d[i], offset=0, ap=[[1, 128], [128, NCH[i]]]),
                )
                g_s.append(gt)
                b_s.append(bt)
            projb_s = wts.tile([128, C1], BF16)
            nc.scalar.dma_start(
                out=projb_s,
                in_=bass.AP(tensor=projb_d, offset=0, ap=[[0, 128], [1, C1]]),
            )

            wkv_s = wts.tile([128, 2, 8, 1024], FP8)
            proj_s = wts.tile([128, 4, C1], BF16)

            def bigw_gen():
                # wkv/proj streamed in chunks through window 0's round-robin
                # so batch-0/1 band DMAs interleave rather than queue behind
                # 2.5 MB of weights.  K-half in kp's consumption order (hi
                # pairs (4,5)/(6,7) first), then V-half, proj last.
                wkvr = wkv_d.ap().rearrange("s (t p) o -> p s t o", p=128)
                for s in range(2):
                    nc.scalar.dma_start(
                        out=wkv_s[:, s, 4:8, 0:512], in_=wkvr[:, s, 4:8, 0:512]
                    )
                    yield
                for s in range(2):
                    nc.scalar.dma_start(
                        out=wkv_s[:, s, 0:4, 0:512], in_=wkvr[:, s, 0:4, 0:512]
                    )
                    yield
                for s in range(2):
                    nc.scalar.dma_start(
                        out=wkv_s[:, s, :, 512:1024], in_=wkvr[:, s, :, 512:1024]
                    )
                    yield
                nc.scalar.dma_start(
                    out=proj_s, in_=proj_d.ap().rearrange("(t p) o -> p t o", p=128)
                )

            def s1_gen(bi, st, pre=None):
                """Loads + PE pooling + conv + bn-stats LN + normalize."""
                x_sb = pre["x_sb"] if pre else load_x(bi)
                # x4 stats via bn_stats (free-axis mean/var per token)
                bst4 = colsp.tile([128, 2, 6], F32, tag="bst4", name="bst4")
                mv4 = colsp.tile([128, 2, 2], F32, tag="mv4", name="mv4")
                for nt in range(2):
                    nc.vector.bn_stats(bst4[:, nt], x_sb[:, nt])
                    nc.vector.bn_aggr(mv4[:, nt], bst4[:, nt])
                yield

                # ---- band loads (fp8) ----
                if pre:
                    vb1, vb2, vb3 = pre["vb1"], pre["vb2"], pre["vb3"]
                else:
                    vb1, vb2, vb3 = load_bands(bi)
                yield

                # x4 cols + xn4
                ln4c = colsp.tile([128, 2], F32, tag="ln4c", name="ln4c")
                nc.scalar.activation(out=ln4c, in_=mv4[:, :, 1], func=AF.Ln, bias=epscol)
                rst4 = colsp.tile([128, 2], F32, tag="rst4", name="rst4")
                nc.scalar.activation(out=rst4, in_=ln4c, func=AF.Exp, scale=-0.5)
                xn4 = work.tile([128, 2, C1], BF16, tag="xn4", bufs=3)
                for nt in range(2):
                    nc.gpsimd.tensor_scalar(
                        xn4[:, nt], x_sb[:, nt],
                        mv4[:, nt, 0:1], rst4[:, nt : nt + 1],
                        ALU.subtract, ALU.mult,
                    )
                st["xn4"] = xn4
                yield

                # x transposes -> xT (fp8); q matmuls (split-fp8 DoubleRow) -> qT
                xT = work.tile([128, 4, NKV], FP8, tag="xT")
                for cp in range(2):
                    tp = pp.tile([128, 2, 2, 128], BF16, tag="ppC", name="xtp", bufs=2)
                    for cl in range(2):
                        ck = cp * 2 + cl
                        for nt in range(2):
                            nc.tensor.transpose(
                                tp[:, cl, nt],
                                x_sb[:, nt, ck * 128 : (ck + 1) * 128],
                                ident,
                            )
                    nc.vector.tensor_copy(
                        xT[:, cp * 2 : (cp + 1) * 2],
                        tp.rearrange("p a b c -> p (a b c)"),
                    )
                    yield
                qT = work.tile([128, 4, NKV], BF16, tag="qT", bufs=4)
                for mp in range(2):
                    qp = pp.tile([128, 2, NKV], F32, tag="ppB", name="qp", bufs=2)
                    for ml in range(2):
                        mt = mp * 2 + ml
                        idx = 0
                        for kpr in range(2):
                            for s in range(2):
                                nc.tensor.matmul(
                                    qp[:, ml],
                                    wq_s[:, s, 2 * kpr : 2 * kpr + 2,
                                         mt * 128 : (mt + 1) * 128],
                                    xT[:, 2 * kpr : 2 * kpr + 2],
                                    start=(idx == 0),
                                    stop=(idx == 3),
                                    perf_mode=PM.DoubleRow,
                                    skip_group_check=True,
                                )
                                idx += 1
                    nc.vector.tensor_copy(
                        qT[:, mp * 2 : (mp + 1) * 2],
                        qp,
                    )
                    yield
                st["qT"] = qT

                # ---- fused w+h-pool on the PE (fp8 DoubleRow over dw pairs)
                poolp1 = pp.tile([64, 16, 16], F32, tag="ppA", name="poolp1", bufs=2)
                for qt in range(4):
                    for wl in range(4):
                        for dp in range(4):
                            nc.tensor.matmul(
                                poolp1[:, qt * 4 + wl],
                                vb1[qt][:, wl, 2 * dp : 2 * dp + 2],
                                ah_s[0],
                                start=(dp == 0),
                                stop=(dp == 3),
                                perf_mode=PM.DoubleRow,
                                skip_group_check=True,
                            )
                poolt1 = pooltp.tile([64, NKV], BF16, tag="poolt1")
                nc.scalar.copy(out=poolt1, in_=poolp1.rearrange("c a b -> c (a b)"))
                yield

                poolp2 = pp.tile([128, 2, 8, 16], F32, tag="ppA", name="poolp2", bufs=2)
                for hf in range(2):
                    for wl in range(4):
                        for dp in range(2):
                            nc.tensor.matmul(
                                poolp2[:, :, hf * 4 + wl],
                                vb2[hf][:, wl, 2 * dp : 2 * dp + 2],
                                ah_s[1],
                                start=(dp == 0),
                                stop=(dp == 1),
                                perf_mode=PM.DoubleRow,
                                skip_group_check=True,
                            )
                poolt2 = pooltp.tile([128, NKV], BF16, tag="poolt2")
                nc.scalar.copy(out=poolt2, in_=poolp2.rearrange("c g a b -> c (g a b)"))
                yield

                poolt3 = pooltp.tile([128, 3, NKV], BF16, tag="poolt3")
                for cs in range(3):
                    cl = 64 if cs == 2 else 128
                    poolp3 = pp.tile([128, 4, 4, 16], F32, tag="ppA", name="poolp3", bufs=2)
                    for hf in range(2):
                        for wl in range(2):
                            nc.tensor.matmul(
                                poolp3[:cl, :, hf * 2 + wl],
                                vb3[hf][:, wl, :, cs * 128 : cs * 128 + cl],
                                ah_s[2],
                                start=True,
                                stop=True,
                                perf_mode=PM.DoubleRow,
                                skip_group_check=True,
                            )
                    nc.scalar.copy(
                        out=poolt3[:cl, cs],
                        in_=poolp3[:cl].rearrange("c g a b -> c (g a b)"),
                    )
                    yield

                # ---- branch conv (token-major) + bn-stats LN + normalize
                poolts = [poolt1, poolt2, poolt3]
                xns = []
                if "conv" in ABLATE:
                    for br in range(3):
                        xn = xnp.tile([128, 2, C2[br]], BF16, tag=f"xn{br}", name=f"xn{br}")
                        nc.vector.memset(xn, 0.2)
                        xns.append(xn)
                    st["xns"] = xns
                    yield
                    return
                xns = [None, None, None]
                for br in [2, 1, 0]:
                    cb = C2[br]
                    nkt = (cb + 127) // 128
                    xn = xnp.tile([128, 2, cb], BF16, tag=f"xn{br}", name=f"xn{br}")
                    bst = colsp.tile([128, 2, 6], F32, tag=f"bst{br}", name=f"bst{br}")
                    mv = colsp.tile([128, 2, 2], F32, tag=f"mv{br}", name=f"mv{br}")
                    lnc = colsp.tile([128, 2], F32, tag=f"ln{br}", name=f"ln{br}")
                    rstd = colsp.tile([128, 2], F32, tag=f"rst{br}", name=f"rst{br}")
                    preps = []
                    for tc in range(2):
                        prep = pp.tile([128, 320], F32, tag="ppA", name=f"prep{br}", bufs=2)
                        nc.tensor.matmul(
                            prep[:, 0:cb],
                            onesrow,
                            srb_s[br],
                            start=True,
                            stop=False,
                        )
                        for kt in range(nkt):
                            kl = min(128, cb - kt * 128)
                            if br < 2:
                                lhs = poolts[br][:kl, tc * 128 : (tc + 1) * 128]
                            else:
                                lhs = poolts[2][:kl, kt, tc * 128 : (tc + 1) * 128]
                            nc.tensor.matmul(
                                prep[:, 0:cb],
                                lhs,
                                srw_s[br][:kl, kt],
                                start=False,
                                stop=(kt == nkt - 1),
                            )
                        nc.vector.bn_stats(bst[:, tc], prep[:, 0:cb])
                        nc.vector.bn_aggr(mv[:, tc], bst[:, tc])
                        preps.append(prep)
                        if tc == 0:
                            yield
                    # one Ln/Exp pair per branch (both tc halves at once)
                    nc.scalar.activation(
                        out=lnc, in_=mv[:, :, 1], func=AF.Ln, bias=epscol
                    )
                    nc.scalar.activation(out=rstd, in_=lnc, func=AF.Exp, scale=-0.5)
                    for tc in range(2):
                        nc.vector.tensor_scalar(
                            xn[:, tc], preps[tc][:, 0:cb],
                            mv[:, tc, 0:1], rstd[:, tc : tc + 1],
                            ALU.subtract, ALU.mult,
                        )
                    yield
                    xns[br] = xn
                st["xns"] = xns

            def s2ab_gen(bi, st):
                """Back-transposes + GELU -> xcT, then kv matmuls
                (split-fp8 DoubleRow) — chunked into the window round-robin."""
                xn4, xns = st["xn4"], st["xns"]
                xcT = work.tile([128, 8, NKV], FP8, tag="xcT")
                for ck in range(4):
                    tp4 = pp.tile([128, 2, 128], BF16, tag="ppC", name="tp4", bufs=2)
                    for nt in range(2):
                        nc.tensor.transpose(
                            tp4[:, nt], xn4[:, nt, ck * 128 : (ck + 1) * 128], ident
                        )
                    dst = xcT[:, 4 + ck]
                    nc.scalar.activation(
                        out=dst.rearrange("c (wo ho) -> c ho wo", wo=16),
                        in_=tp4.rearrange("c nt (hh wo) -> c (nt hh) wo", hh=8),
                        func=AF.Gelu,
                        scale=g_s[3][:, ck : ck + 1],
                        bias=b_s[3][:, ck : ck + 1],
                    )
                    if ck % 2 == 1:
                        yield

                for br in [1, 2, 0]:
                    cb = C2[br]
                    xn = xns[br]
                    for ch in range(NPT[br]):
                        cl = min(128, cb - ch * 128)
                        kt_slot, base = XC_SLOT[br][ch]
                        tpb = pp.tile([128, 2, 128], BF16, tag="ppC", name=f"tpb{br}", bufs=2)
                        for tc in range(2):
                            nc.tensor.transpose(
                                tpb[:cl, tc],
                                xn[:, tc, ch * 128 : ch * 128 + cl],
                                ident,
                            )
                        nc.scalar.activation(
                            out=xcT[base : base + cl, kt_slot],
                            in_=tpb[:cl].rearrange("c a b -> c (a b)"),
                            func=AF.Gelu,
                            scale=g_s[br][0:cl, ch : ch + 1],
                            bias=b_s[br][0:cl, ch : ch + 1],
                        )
                    yield
                st["xcT"] = xcT

                # ---- kv matmuls ----
                # kt pairs in xcT-readiness order: x4 (4..7), then branches
                PAIRS = [4, 6, 2, 0]
                kT = work.tile([128, 4, NKV], BF16, tag="kT")
                if "kv" in ABLATE:
                    nc.vector.memset(kT, 0.1)
                    st["kT"] = kT
                    v_aug = work.tile([128, 2, NH, HD + 1], BF16, tag="v_aug")
                    nc.vector.memset(v_aug, 0.1)
                    st["v_aug"] = v_aug
                    yield
                    return
                for mp in range(2):
                    kp = pp.tile([128, 2, NKV], F32, tag="ppB", name="kp", bufs=2)
                    for ml in range(2):
                        mt = mp * 2 + ml
                        idx = 0
                        for a in PAIRS:
                            for s in range(2):
                                nc.tensor.matmul(
                                    kp[:, ml],
                                    wkv_s[:, s, a : a + 2, mt * 128 : (mt + 1) * 128],
                                    xcT[:, a : a + 2],
                                    start=(idx == 0),
                                    stop=(idx == 7),
                                    perf_mode=PM.DoubleRow,
                                    skip_group_check=True,
                                )
                                idx += 1
                    nc.scalar.copy(out=kT[:, mp * 2 : (mp + 1) * 2], in_=kp)
                    yield
                st["kT"] = kT

                v_aug = work.tile([128, 2, NH, HD + 1], FP8, tag="v_aug")
                nc.vector.memset(v_aug[:, :, :, HD : HD + 1], 1.0)
                for mt in range(2):
                    vp = pp.tile([128, 2, NKV], F32, tag="ppB", name="vp", bufs=2)
                    for vh in range(2):
                        idx = 0
                        for a in PAIRS:
                            for s in range(2):
                                nc.tensor.matmul(
                                    vp[:, vh],
                                    xcT[:, a : a + 2, mt * 128 : (mt + 1) * 128],
                                    wkv_s[:, s, a : a + 2,
                                          512 + vh * 256 : 768 + vh * 256],
                                    start=(idx == 0),
                                    stop=(idx == 7),
                                    perf_mode=PM.DoubleRow,
                                    skip_group_check=True,
                                )
                                idx += 1
                    nc.scalar.copy(
                        out=v_aug[:, mt, :, 0:HD],
                        in_=vp.rearrange("p a (h d) -> p (a h) d", h=4),
                    )
                    yield
                st["v_aug"] = v_aug

            def s3_gen(bi, st):
                """Attention (head-pipelined) + proj + store."""
                qT, kT, v_aug = st["qT"], st["kT"], st["v_aug"]
                outT = work.tile([128, 4, NKV], BF16, tag="outT")
                if "attn" in ABLATE:
                    nc.vector.memset(outT, 0.5)
                    yield
                else:
                    sps, stes, pv2s, rss = {}, {}, {}, {}

                    def emit_sp(h):
                        j, hh = h // 2, h % 2
                        pb = hh * 64
                        sp = pp.tile([128, 2, NKV], F32, tag="ppC", name="sp", bufs=2)
                        for nt in range(2):
                            nc.tensor.matmul(
                                sp[:, nt],
                                kT[pb : pb + 64, j, nt * 128 : (nt + 1) * 128],
                                qT[pb : pb + 64, j],
                                start=True,
                                stop=True,
                                skip_group_check=True,
                            )
                        ste = step.tile([128, 2, NKV], FP8, tag="ste")
                        nc.scalar.activation(
                            out=ste, in_=sp, func=AF.Exp, scale=SCALE / 256.0
                        )
                        stes[h] = ste

                    def emit_pv(h):
                        j, hh = h // 2, h % 2
                        if hh == 0:
                            pv2s[j] = pp.tile([65, 2, NKV], F32, tag="ppD", name="pv2", bufs=2)
                        nc.tensor.matmul(
                            pv2s[j][:, hh],
                            v_aug[:, :, h],
                            stes[h],
                            start=True,
                            stop=True,
                            perf_mode=PM.DoubleRow,
                            skip_group_check=True,
                        )
                        del stes[h]

                    def emit_norm(j):
                        pv2 = pv2s[j]
                        rs2 = rowsp.tile([1, 2, NKV], BF16, tag="rs2")
                        bc = pp.tile([128, NKV], F32, tag="ppB", name="bc", bufs=2)
                        with nc.allow_low_precision(reason="bf16 softmax denom"):
                            nc.vector.reciprocal(rs2, pv2[64:65])
                        nc.tensor.matmul(
                            bc[0:64], onesrow[:, 0:64], rs2[:, 0],
                            start=True, stop=True, skip_group_check=True,
                        )
                        nc.tensor.matmul(
                            bc[64:128], onesrow[:, 0:64], rs2[:, 1],
                            start=True, stop=True, skip_group_check=True,
                        )
                        for hh in range(2):
                            pb = hh * 64
                            nc.vector.scalar_tensor_tensor(
                                out=outT[pb : pb + 64, j],
                                in0=pv2[0:64, hh], scalar=1.0, in1=bc[pb : pb + 64],
                                op0=ALU.mult, op1=ALU.mult,
                            )
                        del pv2s[j]

                    # head-level software pipeline: sp(h+1) issued between
                    # exp(h) and pv(h); pair tails interleave two heads later
                    emit_sp(0)
                    for h in range(NH):
                        if h + 1 < NH:
                            emit_sp(h + 1)
                        emit_pv(h)
                        if h >= 2 and h % 2 == 1:
                            emit_norm(h // 2 - 1)
                            yield
                    emit_norm(3)
                    yield

                osb = work.tile([128, 2, C1], BF16, tag="osb")
                for tc in range(2):
                    fp = pp.tile([128, 2, NKV], F32, tag="ppB", name="fp", bufs=2)
                    kts = [0, 1, 2, 3]
                    for fh in range(2):
                        for kt in kts:
                            nc.tensor.matmul(
                                fp[:, fh],
                                outT[:, kt, tc * 128 : (tc + 1) * 128],
                                proj_s[:, kt, fh * 256 : (fh + 1) * 256],
                                start=(kt == 0),
                                stop=(kt == 3),
                                skip_group_check=True,
                            )
                    # alternate engines so the two bias-adds overlap, and
                    # store each half as soon as it is ready
                    nc.vector.tensor_add(
                        osb[:, tc],
                        fp.rearrange("p a b -> p (a b)"),
                        projb_s,
                    )
                    nc.sync.dma_start(out=outr[bi][:, tc], in_=osb[:, tc])
                    yield

            def _drain(g):
                if g is None:
                    return False
                try:
                    next(g)
                    return True
                except StopIteration:
                    return False

            # ---- software pipeline ------------------------------------
            # Window t round-robins chunks of S3(t-2) / S2ab(t-1) / S1y(t) /
            # S1x(t+1); window 0 also streams the big weights between band
            # DMAs and runs S1x(0).
            NB = reps * BPC
            states = {}
            for t in range(NB + 2):
                gens = []
                if t < NB:
                    states[t] = {}
                if t >= 2:
                    gens.append(s3_gen((t - 2) % BPC, states[t - 2]))
                if 1 <= t and t - 1 < NB:
                    gens.append(s2ab_gen((t - 1) % BPC, states[t - 1]))
                if t < NB:
                    gens.append(s1_gen(t % BPC, states[t],
                                       pre=pre0 if t == 0 else None))
                if t == 0:
                    gens.append(bigw_gen())
                while gens:
                    nxt = []
                    for g in gens:
                        try:
                            next(g)
                            nxt.append(g)
                        except StopIteration:
                            pass
                    gens = nxt
                if t >= 2:
                    del states[t - 2]

    _split_excess_waits(nc)
    return nc


def _split_fp8(w16):
    """w16 (f32) -> stacked [2, ...] fp8 hi/lo with hi+lo ~= w16."""
    f8 = ml_dtypes.float8_e4m3
    hi = w16.astype(f8)
    lo = (w16 - hi.astype(np.float32)).astype(f8)
    return np.stack([hi, lo], axis=0)


def _prep_common(inputs):
    Wq = np.asarray(inputs["Wq"], dtype=np.float32)
    Wkv = np.asarray(inputs["Wkv"], dtype=np.float32)
    proj_w = np.asarray(inputs["proj_w"], dtype=np.float32)
    proj_b = np.asarray(inputs["proj_b"], dtype=np.float32)

    bf = ml_dtypes.bfloat16
    f8 = ml_dtypes.float8_e4m3
    common = {
        # x16 prescale keeps the fp8 split residual out of subnormals; the
        # 16*16=256 score scale folds into the Exp activation, the 16x on v
        # folds into proj_t below.
        "wq_t": _split_fp8(np.ascontiguousarray(Wq.T) * 16.0),
        "wkv_t": _split_fp8(np.ascontiguousarray(Wkv.T[_PERM, :]) * 16.0),
        "proj_t": (np.ascontiguousarray(proj_w.T) / 16.0).astype(bf),
        "projb": proj_b.astype(bf),
    }
    ah = _pool_mats()
    for i in range(3):
        common[f"ah{i+1}"] = np.stack([ah[i], ah[i]], axis=1).astype(f8)
        c = C2[i]
        cpad = ((c + 127) // 128) * 128
        pr = min(c, 128)
        nkt = (c + 127) // 128
        srw_t = np.asarray(inputs[f"sr{i+1}_w"], dtype=np.float32).T  # [c_in, c_out]
        srw_p = np.zeros((nkt * pr, c), dtype=np.float32)
        srw_p[:c] = srw_t
        common[f"srw{i+1}_t"] = srw_p.astype(bf)
        common[f"srb{i+1}"] = np.asarray(
            inputs[f"sr{i+1}_b"], dtype=np.float32
        ).astype(bf)
    for i, c in enumerate((64, 128, 320, 512)):
        cpad = ((c + 127) // 128) * 128
        if i < 3:
            g = np.asarray(inputs[f"ln{i+1}_g"], dtype=np.float32)
            b = np.asarray(inputs[f"ln{i+1}_b"], dtype=np.float32)
        else:
            g = np.asarray(inputs["ln4_g"], dtype=np.float32)
            b = np.asarray(inputs["ln4_b"], dtype=np.float32)
        gp = np.zeros(cpad, dtype=np.float32)
        gp[:c] = g
        bp = np.zeros(cpad, dtype=np.float32)
        bp[:c] = b
        common[f"g{i+1}"] = gp
        common[f"lb{i+1}"] = bp
    return common


def kernel(**inputs):
    bf = ml_dtypes.bfloat16
    f8 = ml_dtypes.float8_e4m3
    x = np.ascontiguousarray(inputs["x"]).astype(bf)
    y1 = np.ascontiguousarray(inputs["y1"]).astype(f8)
    y2 = np.ascontiguousarray(inputs["y2"]).astype(f8)
    y3 = np.ascontiguousarray(inputs["y3"]).astype(f8)
    common = _prep_common(inputs)

    nc = build_module()
    in_maps = []
    for c in range(NCORES):
        sl = slice(c * BPC, (c + 1) * BPC)
        m = dict(common)
        m["x"] = x[sl]
        m["y1"] = y1[sl]
        m["y2"] = y2[sl]
        m["y3"] = y3[sl]
        in_maps.append(m)

    res = run_bass_kernel_spmd(nc, in_maps, core_ids=list(range(NCORES)))
    return np.concatenate(
        [np.asarray(r["out"]).astype(np.float32) for r in res.results], axis=0
    )


if __name__ == "__main__":
    pass



# revision 54
# speedup vs baseline: 1.0520x; 1.0505x over previous
"""Trainium2 Bass kernel for nn_MultiCrossAttention (PVT-style multi-scale
spatial-reduction cross attention) — v2.

Sharding: data-parallel over batch (B=32 -> 4 per core x 8 cores), weights
replicated.  All inputs are cast to bf16 on the host (tolerance is 2e-2; bf16
keeps us ~5e-3) which halves HBM traffic — the memory roofline.

Per-batch pipeline:
  y_i --(contig band DMA, bf16)--> w-pool tree (DVE adds) -> fused
  h-pool+transpose matmuls (PE, pool matrix Ah) -> poolT [c,256] (chan-major).
  Conv runs TOKEN-major: out[tok, c_out] = poolT-chunk^T @ srwT-chunk (+bias
  via K=1 ones-row matmul).  LN stats are then free-axis reductions
  (tensor_reduce / stt accum_out) giving per-token mean/var COLUMNS;
  rstd = exp(-0.5*ln(var+eps)) on the Act engine (Ln+Exp share one
  activation table with the attention Exp — 2 table loads per batch).
  Normalize = (conv - m)*rstd via per-partition tensor_scalar, transpose
  back to chan-major on the PE, and GELU reads the transpose PSUM directly
  with gamma/beta folded into the Act op's per-partition scale/bias.
  x: PE transpose -> xT -> q matmuls; x4 branch same token-major LN trick.
  kv matmuls -> kT (chan-major) + v_aug (token-major, ones column for the
  softmax denominator).  Scores TRANSPOSED (sT[kv,q]) so the denominator
  falls out of the PV matmul's 65th row; normalization via reciprocal +
  rank-1 ones2 broadcast (two heads per matmul) + fused scalar_tensor_tensor.
  proj matmuls (token-major) -> + bias -> out.
"""

import sys

sys.path.insert(0, "/opt/trn_rl_repo")

import numpy as np
import ml_dtypes

import concourse.bass as bass
import concourse.mybir as mybir
import concourse.tile as tile
from concourse.bass_utils import run_bass_kernel_spmd
from concourse.masks import make_identity

# ---------------------------------------------------------------------------
# Patch: this walrus build only accepts ONE sync-wait on a Drain instruction;
# Tile's tail drain waits on every live semaphore lane.  Split it into a chain
# of single-wait drains.
from concourse.vector_clock import ScopedClock, VectorClock
from concourse.tile_sem_assignment import N_PROCS


def _patched_drain_and_barrier(self, tick_clock, wait_clock):
    # Walrus accepts only ONE sync-wait per Drain; instead of a serial chain
    # of single-wait drains on SP, spread them across all five engine queues
    # so the lane waits resolve in parallel, then barrier.
    nc = self.nc
    gc = tick_clock.global_clock
    nz = [p for p in range(N_PROCS) if gc[p] > 0]
    engines = [nc.sync, nc.scalar, nc.vector, nc.gpsimd, nc.tensor]
    for i, p in enumerate(nz):
        masked = VectorClock([gc[q] if q == p else 0 for q in range(N_PROCS)])
        d = engines[i % len(engines)].drain()
        wait_clock.add_sem_waits(d.ins, ScopedClock({None: masked}))
    if not nz:
        nc.sync.drain()
    nc.all_engine_barrier()
    assert self.sems is not None
    popped = nc._tile_sem_poison_stack.pop()
    assert popped is self._sem_poison
    nc.clear_and_free_semaphores(list(self.sems.allocated().values()))
    nc.all_engine_barrier()


tile.TileContext._drain_and_barrier = _patched_drain_and_barrier


def _split_excess_waits(nc, limit=1):
    """Walrus in this build rejects >2 sync-waits on compute/DMA instructions
    (>1 on Drain).  Move excess waits onto same-engine no-ops inserted just
    before the offending instruction."""
    import bass_rust

    uid = [0]
    for f in nc.m.functions:
        for blk in f.blocks:
            newlist = []
            changed = False
            for ins in blk.instructions:
                si = ins.sync_info
                waits = list(si.on_wait) if si and si.on_wait else []
                tn = type(ins).__name__
                lim = 1 if tn in ("InstDrain", "InstNoOp", "InstTensorTensor") else limit
                if len(waits) > lim:
                    keep = waits[-lim:]
                    for w in waits[:-lim]:
                        nop = bass_rust.InstNoOp(
                            name=f"wsplit-{uid[0]}", ins=[], outs=[]
                        )
                        uid[0] += 1
                        nop.engine = ins.engine
                        nop.sync_info = mybir.SyncInfo(on_wait=[w], on_update=[])
                        newlist.append(nop)
                    ins.sync_info = mybir.SyncInfo(
                        on_wait=keep,
                        on_update=list(si.on_update) if si.on_update else [],
                    )
                    changed = True
                newlist.append(ins)
            if changed:
                blk.instructions = newlist


# ---------------------------------------------------------------------------

F32 = mybir.dt.float32
BF16 = mybir.dt.bfloat16
FP8 = mybir.dt.float8e4
PM = mybir.MatmulPerfMode
AF = mybir.ActivationFunctionType
ALU = mybir.AluOpType

NCORES = 8
B = 32
BPC = B // NCORES  # batches per core
N1 = 256  # query tokens
C1 = 512
NH, HD = 8, 64
SCALE = HD ** -0.5
EPS = 1e-5
C2 = (64, 128, 320)
RATIO = (8, 4, 2)
GRP = (1, 2, 4)  # w-groups packed into partitions (128 = H*G)
NKV = 256  # kv tokens (16x16 pooled grid for every branch)

# xc channel-permutation: kt bins of 128 rows; each branch ptile lands at a
# 64-aligned partition base.  Global xc order: x1 0:64 | x2 64:192 | x3
# 192:512 | x4 512:1024.
# kt0=[x1 | x3c], kt1=x2, kt2=x3a, kt3=x3b, kt4..7=x4
_PERM = np.concatenate(
    [
        np.arange(0, 64),  # x1        -> kt0[0:64]
        np.arange(448, 512),  # x3 pt2  -> kt0[64:128]
        np.arange(64, 192),  # x2       -> kt1
        np.arange(192, 320),  # x3 pt0  -> kt2
        np.arange(320, 448),  # x3 pt1  -> kt3
        np.arange(512, 1024),  # x4     -> kt4..7
    ]
)


def _pool_mats():
    """Ah matrices: [128, G*16] mapping partition (h,g) -> col (g*16+ho),
    with the full 1/r^2 divisor folded in."""
    out = []
    for i in range(3):
        G, r = GRP[i], RATIO[i]
        H = 128 // G
        m = np.zeros((128, G * 16), dtype=np.float32)
        for h in range(H):
            for g in range(G):
                p = h * G + g
                ho = h // r
                m[p, g * 16 + ho] = 1.0 / (r * r)
        out.append(m)
    return out


ABLATE = set()


def build_module(reps=1):
    nc = bass.Bass(trn_type="TRN2")

    # ---- DRAM I/O -------------------------------------------------------
    x_d = nc.dram_tensor("x", [BPC, N1, C1], BF16, kind="ExternalInput")
    y1_d = nc.dram_tensor("y1", [BPC, 128 * 128, 64], FP8, kind="ExternalInput")
    y2_d = nc.dram_tensor("y2", [BPC, 64 * 64, 128], FP8, kind="ExternalInput")
    y3_d = nc.dram_tensor("y3", [BPC, 32 * 32, 320], FP8, kind="ExternalInput")
    # split-fp8 weights: W*16 = hi + lo (hi/lo stacked on a leading dim)
    wq_d = nc.dram_tensor("wq_t", [2, C1, C1], FP8, kind="ExternalInput")
    wkv_d = nc.dram_tensor("wkv_t", [2, 1024, 1024], FP8, kind="ExternalInput")
    proj_d = nc.dram_tensor("proj_t", [C1, C1], BF16, kind="ExternalInput")
    projb_d = nc.dram_tensor("projb", [C1], BF16, kind="ExternalInput")
    # small weights packed host-side into one tensor per dtype: cuts ~16
    # scalar-queue DMAs (625ns HWDGE dispatch each) down to 3.
    # wbp (bf16) [128, 2176]: srw1 0:64 (rows 0:64) | srw2 64:192 |
    #   srw3 192:1152 (3 kt x 320) | srb row 1152:1664 (partition 0:
    #   srb1 0:64, srb2 64:192, srb3 192:512) | projb 1664:2176
    # ahp (fp8) [128, 224]: ah1 0:32 | ah2 32:96 | ah3 96:224
    # gbp (f32) [128, 18]: g1|b1|g2|b2|g3(3)|b3(3)|g4(4)|b4(4)
    CB = [64, 128, 320, 512]  # channels per branch (incl. x4)
    NCH = [1, 1, 3, 4]  # 128-channel chunks per branch
    wbp_d = nc.dram_tensor("wbp", [128, 2176], BF16, kind="ExternalInput")
    ahp_d = nc.dram_tensor("ahp", [128, 224], FP8, kind="ExternalInput")
    gbp_d = nc.dram_tensor("gbp", [128, 18], F32, kind="ExternalInput")
    out_d = nc.dram_tensor("out", [BPC, N1, C1], BF16, kind="ExternalOutput")

    NPT = [1, 1, 3, 4]  # partition tiles per branch in xcT
    # (kt, base) of each branch ptile in xcT
    XC_SLOT = {
        0: [(0, 0)],
        1: [(1, 0)],
        2: [(2, 0), (3, 0), (0, 64)],
        3: [(4, 0), (5, 0), (6, 0), (7, 0)],
    }

    with tile.TileContext(nc) as tc:
        with (
            tc.tile_pool(name="wts", bufs=1) as wts,
            tc.tile_pool(name="bands", bufs=2) as bandp,
            tc.tile_pool(name="t1", bufs=2) as t1p,
            tc.tile_pool(name="poolt", bufs=2) as pooltp,
            tc.tile_pool(name="work", bufs=2) as work,
            tc.tile_pool(name="xn", bufs=2) as xnp,
            tc.tile_pool(name="scrap", bufs=4) as scrapp,
            tc.tile_pool(name="cols", bufs=2) as colsp,
            tc.tile_pool(name="rows", bufs=2) as rowsp,
            tc.tile_pool(name="ste", bufs=2) as step,
            tc.tile_pool(name="pp", bufs=1, space="PSUM") as pp,
        ):
            y1r = y1_d.ap().rearrange("b (h w) c -> b h (w c)", h=128)
            y2r = y2_d.ap().rearrange("b (h wb wi) c -> b (h wb) (wi c)", wb=2, wi=32)
            y3r = y3_d.ap().rearrange("b (h wb wi) c -> b (h wb) (wi c)", wb=4, wi=8)
            xr = x_d.ap().rearrange("b (nt p) c -> b p nt c", p=128)
            outr = out_d.ap().rearrange("b (nt p) c -> b p nt c", p=128)

            def load_x(bi):
                x_sb = work.tile([128, 2, C1], BF16, tag="x_sb")
                nc.sync.dma_start(out=x_sb, in_=xr[bi])
                return x_sb

            def load_bands(bi, split_y1=False):
                """One DMA per y tensor (HWDGE dispatch is ~625ns/DMA and a
                co-bottleneck); split_y1 chunks y1 so batch-0 pooling can
                start on the first quarter."""
                b1 = bandp.tile([128, 4, 2048], FP8, tag="band1", name="band1",
                                bufs=2)
                if split_y1:
                    for qt in range(4):
                        nc.sync.dma_start(
                            out=b1[:, qt],
                            in_=y1r[bi, :, qt * 2048 : (qt + 1) * 2048],
                        )
                else:
                    nc.sync.dma_start(
                        out=b1, in_=y1r[bi].rearrange("p (q x) -> p q x", q=4)
                    )
                b2 = bandp.tile([128, 2, 2048], FP8, tag="band2", name="band2",
                                bufs=2)
                nc.sync.dma_start(
                    out=b2, in_=y2r[bi].rearrange("p (q x) -> p q x", q=2)
                )
                b3 = bandp.tile([128, 2, 1280], FP8, tag="band3", name="band3",
                                bufs=2)
                nc.sync.dma_start(
                    out=b3, in_=y3r[bi].rearrange("p (q x) -> p q x", q=2)
                )
                vb1 = [b1[:, qt].rearrange("p (wo dw c) -> p wo dw c", wo=4, dw=8)
                       for qt in range(4)]
                vb2 = [b2[:, hf].rearrange("p (wo dw c) -> p wo dw c", wo=4, dw=4)
                       for hf in range(2)]
                vb3 = [b3[:, hf].rearrange("p (wo dw c) -> p wo dw c", wo=2, dw=2)
                       for hf in range(2)]
                return vb1, vb2, vb3

            # ---- batch-0 input DMAs lead the queue (PE's first dependencies)
            pre0 = {"x_sb": load_x(0)}
            pre0["vb1"], pre0["vb2"], pre0["vb3"] = load_bands(0, split_y1=True)

            # ---- non-DMA constants
            ident = wts.tile([128, 128], BF16)
            make_identity(nc, ident)
            onesrow = wts.tile([1, 128], BF16)
            nc.vector.memset(onesrow, 1.0)
            epscol = wts.tile([128, 1], F32)
            nc.gpsimd.memset(epscol, EPS)

            # ---- weights in first-use order: ah (pool) -> wq (q) -> conv/LN
            ahp_s = wts.tile([128, 224], FP8)
            nc.scalar.dma_start(out=ahp_s, in_=ahp_d.ap())
            ah_s = [
                ahp_s[:, 0:32].rearrange("p (a b) -> p a b", a=2),
                ahp_s[:, 32:96].rearrange("p (a b) -> p a b", a=2),
                ahp_s[:, 96:224].rearrange("p (a b) -> p a b", a=2),
            ]
            wq_s = wts.tile([128, 2, 4, C1], FP8)
            nc.scalar.dma_start(
                out=wq_s, in_=wq_d.ap().rearrange("s (t p) o -> p s t o", p=128)
            )
            wbp_s = wts.tile([128, 2176], BF16)
            nc.scalar.dma_start(out=wbp_s, in_=wbp_d.ap())
            srw_s = [
                wbp_s[0:64, 0:64].rearrange("p (t c) -> p t c", t=1),
                wbp_s[:, 64:192].rearrange("p (t c) -> p t c", t=1),
                wbp_s[:, 192:1152].rearrange("p (t c) -> p t c", t=3),
            ]
            srb_s = [
                wbp_s[0:1, 1152:1216],
                wbp_s[0:1, 1216:1344],
                wbp_s[0:1, 1344:1664],
            ]
            projb_s = wbp_s[:, 1664:2176]
            gbp_s = wts.tile([128, 18], F32)
            nc.scalar.dma_start(out=gbp_s, in_=gbp_d.ap())
            g_s = [gbp_s[:, 0:1], gbp_s[:, 2:3], gbp_s[:, 4:7], gbp_s[:, 10:14]]
            b_s = [gbp_s[:, 1:2], gbp_s[:, 3:4], gbp_s[:, 7:10], gbp_s[:, 14:18]]

            wkv_s = wts.tile([128, 2, 8, 1024], FP8)
            proj_s = wts.tile([128, 4, C1], BF16)

            def bigw_gen():
                # wkv/proj streamed in chunks through window 0's round-robin
                # so batch-0/1 band DMAs interleave rather than queue behind
                # 2.5 MB of weights.  K-half in kp's consumption order (hi
                # pairs (4,5)/(6,7) first), then V-half, proj last.
                wkvr = wkv_d.ap().rearrange("s (t p) o -> p s t o", p=128)
                for s in range(2):
                    nc.scalar.dma_start(
                        out=wkv_s[:, s, 4:8, 0:512], in_=wkvr[:, s, 4:8, 0:512]
                    )
                    yield
                for s in range(2):
                    nc.scalar.dma_start(
                        out=wkv_s[:, s, 0:4, 0:512], in_=wkvr[:, s, 0:4, 0:512]
                    )
                    yield
                for s in range(2):
                    nc.scalar.dma_start(
                        out=wkv_s[:, s, :, 512:1024], in_=wkvr[:, s, :, 512:1024]
                    )
                    yield
                nc.scalar.dma_start(
                    out=proj_s, in_=proj_d.ap().rearrange("(t p) o -> p t o", p=128)
                )

            def s1_gen(bi, st, pre=None):
                """Loads + PE pooling + conv + bn-stats LN + normalize."""
                x_sb = pre["x_sb"] if pre else load_x(bi)
                # x4 stats via bn_stats (free-axis mean/var per token)
                bst4 = colsp.tile([128, 2, 6], F32, tag="bst4", name="bst4")
                mv4 = colsp.tile([128, 2, 2], F32, tag="mv4", name="mv4")
                for nt in range(2):
                    nc.vector.bn_stats(bst4[:, nt], x_sb[:, nt])
                    nc.vector.bn_aggr(mv4[:, nt], bst4[:, nt])
                yield

                # ---- band loads (fp8) ----
                if pre:
                    vb1, vb2, vb3 = pre["vb1"], pre["vb2"], pre["vb3"]
                else:
                    vb1, vb2, vb3 = load_bands(bi)
                yield

                # x4 cols + xn4
                ln4c = colsp.tile([128, 2], F32, tag="ln4c", name="ln4c")
                nc.scalar.activation(out=ln4c, in_=mv4[:, :, 1], func=AF.Ln, bias=epscol)
                rst4 = colsp.tile([128, 2], F32, tag="rst4", name="rst4")
                nc.scalar.activation(out=rst4, in_=ln4c, func=AF.Exp, scale=-0.5)
                xn4 = work.tile([128, 2, C1], BF16, tag="xn4", bufs=3)
                for nt in range(2):
                    nc.gpsimd.tensor_scalar(
                        xn4[:, nt], x_sb[:, nt],
                        mv4[:, nt, 0:1], rst4[:, nt : nt + 1],
                        ALU.subtract, ALU.mult,
                    )
                st["xn4"] = xn4
                yield

                # x transposes -> xT (fp8); q matmuls (split-fp8 DoubleRow) -> qT
                xT = work.tile([128, 4, NKV], FP8, tag="xT")
                for cp in range(2):
                    tp = pp.tile([128, 2, 2, 128], BF16, tag="ppC", name="xtp", bufs=2)
                    for cl in range(2):
                        ck = cp * 2 + cl
                        for nt in range(2):
                            nc.tensor.transpose(
                                tp[:, cl, nt],
                                x_sb[:, nt, ck * 128 : (ck + 1) * 128],
                                ident,
                            )
                    nc.vector.tensor_copy(
                        xT[:, cp * 2 : (cp + 1) * 2],
                        tp.rearrange("p a b c -> p (a b c)"),
                    )
                    yield
                qT = work.tile([128, 4, NKV], BF16, tag="qT", bufs=4)
                for mp in range(2):
                    qp = pp.tile([128, 2, NKV], F32, tag="ppB", name="qp", bufs=2)
                    for ml in range(2):
                        mt = mp * 2 + ml
                        idx = 0
                        for kpr in range(2):
                            for s in range(2):
                                nc.tensor.matmul(
                                    qp[:, ml],
                                    wq_s[:, s, 2 * kpr : 2 * kpr + 2,
                                         mt * 128 : (mt + 1) * 128],
                                    xT[:, 2 * kpr : 2 * kpr + 2],
                                    start=(idx == 0),
                                    stop=(idx == 3),
                                    perf_mode=PM.DoubleRow,
                                    skip_group_check=True,
                                )
                                idx += 1
                    nc.vector.tensor_copy(
                        qT[:, mp * 2 : (mp + 1) * 2],
                        qp,
                    )
                    yield
                st["qT"] = qT

                # ---- fused w+h-pool on the PE (fp8 DoubleRow over dw pairs)
                poolp1 = pp.tile([64, 16, 16], F32, tag="ppA", name="poolp1", bufs=2)
                for qt in range(4):
                    for wl in range(4):
                        for dp in range(4):
                            nc.tensor.matmul(
                                poolp1[:, qt * 4 + wl],
                                vb1[qt][:, wl, 2 * dp : 2 * dp + 2],
                                ah_s[0],
                                start=(dp == 0),
                                stop=(dp == 3),
                                perf_mode=PM.DoubleRow,
                                skip_group_check=True,
                            )
                poolt1 = pooltp.tile([64, NKV], BF16, tag="poolt1")
                nc.scalar.copy(out=poolt1, in_=poolp1.rearrange("c a b -> c (a b)"))
                yield

                poolp2 = pp.tile([128, 2, 8, 16], F32, tag="ppA", name="poolp2", bufs=2)
                for hf in range(2):
                    for wl in range(4):
                        for dp in range(2):
                            nc.tensor.matmul(
                                poolp2[:, :, hf * 4 + wl],
                                vb2[hf][:, wl, 2 * dp : 2 * dp + 2],
                                ah_s[1],
                                start=(dp == 0),
                                stop=(dp == 1),
                                perf_mode=PM.DoubleRow,
                                skip_group_check=True,
                            )
                poolt2 = pooltp.tile([128, NKV], BF16, tag="poolt2")
                nc.scalar.copy(out=poolt2, in_=poolp2.rearrange("c g a b -> c (g a b)"))
                yield

                poolt3 = pooltp.tile([128, 3, NKV], BF16, tag="poolt3")
                for cs in range(3):
                    cl = 64 if cs == 2 else 128
                    poolp3 = pp.tile([128, 4, 4, 16], F32, tag="ppA", name="poolp3", bufs=2)
                    for hf in range(2):
                        for wl in range(2):
                            nc.tensor.matmul(
                                poolp3[:cl, :, hf * 2 + wl],
                                vb3[hf][:, wl, :, cs * 128 : cs * 128 + cl],
                                ah_s[2],
                                start=True,
                                stop=True,
                                perf_mode=PM.DoubleRow,
                                skip_group_check=True,
                            )
                    nc.scalar.copy(
                        out=poolt3[:cl, cs],
                        in_=poolp3[:cl].rearrange("c g a b -> c (g a b)"),
                    )
                    yield

                # ---- branch conv (token-major) + bn-stats LN + normalize
                poolts = [poolt1, poolt2, poolt3]
                xns = []
                if "conv" in ABLATE:
                    for br in range(3):
                        xn = xnp.tile([128, 2, C2[br]], BF16, tag=f"xn{br}", name=f"xn{br}")
                        nc.vector.memset(xn, 0.2)
                        xns.append(xn)
                    st["xns"] = xns
                    yield
                    return
                xns = [None, None, None]
                for br in [2, 1, 0]:
                    cb = C2[br]
                    nkt = (cb + 127) // 128
                    xn = xnp.tile([128, 2, cb], BF16, tag=f"xn{br}", name=f"xn{br}")
                    bst = colsp.tile([128, 2, 6], F32, tag=f"bst{br}", name=f"bst{br}")
                    mv = colsp.tile([128, 2, 2], F32, tag=f"mv{br}", name=f"mv{br}")
                    lnc = colsp.tile([128, 2], F32, tag=f"ln{br}", name=f"ln{br}")
                    rstd = colsp.tile([128, 2], F32, tag=f"rst{br}", name=f"rst{br}")
                    preps = []
                    for tc in range(2):
                        prep = pp.tile([128, 320], F32, tag="ppA", name=f"prep{br}", bufs=2)
                        nc.tensor.matmul(
                            prep[:, 0:cb],
                            onesrow,
                            srb_s[br],
                            start=True,
                            stop=False,
                        )
                        for kt in range(nkt):
                            kl = min(128, cb - kt * 128)
                            if br < 2:
                                lhs = poolts[br][:kl, tc * 128 : (tc + 1) * 128]
                            else:
                                lhs = poolts[2][:kl, kt, tc * 128 : (tc + 1) * 128]
                            nc.tensor.matmul(
                                prep[:, 0:cb],
                                lhs,
                                srw_s[br][:kl, kt],
                                start=False,
                                stop=(kt == nkt - 1),
                            )
                        nc.vector.bn_stats(bst[:, tc], prep[:, 0:cb])
                        nc.vector.bn_aggr(mv[:, tc], bst[:, tc])
                        preps.append(prep)
                        if tc == 0:
                            yield
                    # one Ln/Exp pair per branch (both tc halves at once)
                    nc.scalar.activation(
                        out=lnc, in_=mv[:, :, 1], func=AF.Ln, bias=epscol
                    )
                    nc.scalar.activation(out=rstd, in_=lnc, func=AF.Exp, scale=-0.5)
                    for tc in range(2):
                        nc.vector.tensor_scalar(
                            xn[:, tc], preps[tc][:, 0:cb],
                            mv[:, tc, 0:1], rstd[:, tc : tc + 1],
                            ALU.subtract, ALU.mult,
                        )
                    yield
                    xns[br] = xn
                st["xns"] = xns

            def s2ab_gen(bi, st):
                """Back-transposes + GELU -> xcT, then kv matmuls
                (split-fp8 DoubleRow) — chunked into the window round-robin."""
                xn4, xns = st["xn4"], st["xns"]
                xcT = work.tile([128, 8, NKV], FP8, tag="xcT")
                for ck in range(4):
                    tp4 = pp.tile([128, 2, 128], BF16, tag="ppC", name="tp4", bufs=2)
                    for nt in range(2):
                        nc.tensor.transpose(
                            tp4[:, nt], xn4[:, nt, ck * 128 : (ck + 1) * 128], ident
                        )
                    dst = xcT[:, 4 + ck]
                    nc.scalar.activation(
                        out=dst.rearrange("c (wo ho) -> c ho wo", wo=16),
                        in_=tp4.rearrange("c nt (hh wo) -> c (nt hh) wo", hh=8),
                        func=AF.Gelu,
                        scale=g_s[3][:, ck : ck + 1],
                        bias=b_s[3][:, ck : ck + 1],
                    )
                    if ck % 2 == 1:
                        yield

                for br in [1, 2, 0]:
                    cb = C2[br]
                    xn = xns[br]
                    for ch in range(NPT[br]):
                        cl = min(128, cb - ch * 128)
                        kt_slot, base = XC_SLOT[br][ch]
                        tpb = pp.tile([128, 2, 128], BF16, tag="ppC", name=f"tpb{br}", bufs=2)
                        for tc in range(2):
                            nc.tensor.transpose(
                                tpb[:cl, tc],
                                xn[:, tc, ch * 128 : ch * 128 + cl],
                                ident,
                            )
                        nc.scalar.activation(
                            out=xcT[base : base + cl, kt_slot],
                            in_=tpb[:cl].rearrange("c a b -> c (a b)"),
                            func=AF.Gelu,
                            scale=g_s[br][0:cl, ch : ch + 1],
                            bias=b_s[br][0:cl, ch : ch + 1],
                        )
                    yield
                st["xcT"] = xcT

                # ---- kv matmuls ----
                # kt pairs in xcT-readiness order: x4 (4..7), then branches
                PAIRS = [4, 6, 2, 0]
                kT = work.tile([128, 4, NKV], BF16, tag="kT")
                if "kv" in ABLATE:
                    nc.vector.memset(kT, 0.1)
                    st["kT"] = kT
                    v_aug = work.tile([128, 2, NH, HD + 1], BF16, tag="v_aug")
                    nc.vector.memset(v_aug, 0.1)
                    st["v_aug"] = v_aug
                    yield
                    return
                for mp in range(2):
                    kp = pp.tile([128, 2, NKV], F32, tag="ppB", name="kp", bufs=2)
                    for ml in range(2):
                        mt = mp * 2 + ml
                        idx = 0
                        for a in PAIRS:
                            for s in range(2):
                                nc.tensor.matmul(
                                    kp[:, ml],
                                    wkv_s[:, s, a : a + 2, mt * 128 : (mt + 1) * 128],
                                    xcT[:, a : a + 2],
                                    start=(idx == 0),
                                    stop=(idx == 7),
                                    perf_mode=PM.DoubleRow,
                                    skip_group_check=True,
                                )
                                idx += 1
                    nc.scalar.copy(out=kT[:, mp * 2 : (mp + 1) * 2], in_=kp)
                    yield
                st["kT"] = kT

                v_aug = work.tile([128, 2, NH, HD + 1], FP8, tag="v_aug")
                nc.vector.memset(v_aug[:, :, :, HD : HD + 1], 1.0)
                for mt in range(2):
                    vp = pp.tile([128, 2, NKV], F32, tag="ppB", name="vp", bufs=2)
                    for vh in range(2):
                        idx = 0
                        for a in PAIRS:
                            for s in range(2):
                                nc.tensor.matmul(
                                    vp[:, vh],
                                    xcT[:, a : a + 2, mt * 128 : (mt + 1) * 128],
                                    wkv_s[:, s, a : a + 2,
                                          512 + vh * 256 : 768 + vh * 256],
                                    start=(idx == 0),
                                    stop=(idx == 7),
                                    perf_mode=PM.DoubleRow,
                                    skip_group_check=True,
                                )
                                idx += 1
                    nc.scalar.copy(
                        out=v_aug[:, mt, :, 0:HD],
                        in_=vp.rearrange("p a (h d) -> p (a h) d", h=4),
                    )
                    yield
                st["v_aug"] = v_aug

            def s3_gen(bi, st):
                """Attention (head-pipelined) + proj + store."""
                qT, kT, v_aug = st["qT"], st["kT"], st["v_aug"]
                outT = work.tile([128, 4, NKV], BF16, tag="outT")
                if "attn" in ABLATE:
                    nc.vector.memset(outT, 0.5)
                    yield
                else:
                    sps, stes, pv2s, rss = {}, {}, {}, {}

                    def emit_sp(h):
                        j, hh = h // 2, h % 2
                        pb = hh * 64
                        sp = pp.tile([128, 2, NKV], F32, tag="ppC", name="sp", bufs=2)
                        for nt in range(2):
                            nc.tensor.matmul(
                                sp[:, nt],
                                kT[pb : pb + 64, j, nt * 128 : (nt + 1) * 128],
                                qT[pb : pb + 64, j],
                                start=True,
                                stop=True,
                                skip_group_check=True,
                            )
                        ste = step.tile([128, 2, NKV], FP8, tag="ste")
                        nc.scalar.activation(
                            out=ste, in_=sp, func=AF.Exp, scale=SCALE / 256.0
                        )
                        stes[h] = ste

                    def emit_pv(h):
                        j, hh = h // 2, h % 2
                        if hh == 0:
                            pv2s[j] = pp.tile([65, 2, NKV], F32, tag="ppD", name="pv2", bufs=2)
                        nc.tensor.matmul(
                            pv2s[j][:, hh],
                            v_aug[:, :, h],
                            stes[h],
                            start=True,
                            stop=True,
                            perf_mode=PM.DoubleRow,
                            skip_group_check=True,
                        )
                        del stes[h]

                    def emit_norm(j):
                        pv2 = pv2s[j]
                        rs2 = rowsp.tile([1, 2, NKV], BF16, tag="rs2")
                        bc = pp.tile([128, NKV], F32, tag="ppB", name="bc", bufs=2)
                        with nc.allow_low_precision(reason="bf16 softmax denom"):
                            nc.vector.reciprocal(rs2, pv2[64:65])
                        nc.tensor.matmul(
                            bc[0:64], onesrow[:, 0:64], rs2[:, 0],
                            start=True, stop=True, skip_group_check=True,
                        )
                        nc.tensor.matmul(
                            bc[64:128], onesrow[:, 0:64], rs2[:, 1],
                            start=True, stop=True, skip_group_check=True,
                        )
                        for hh in range(2):
                            pb = hh * 64
                            nc.vector.scalar_tensor_tensor(
                                out=outT[pb : pb + 64, j],
                                in0=pv2[0:64, hh], scalar=1.0, in1=bc[pb : pb + 64],
                                op0=ALU.mult, op1=ALU.mult,
                            )
                        del pv2s[j]

                    # head-level software pipeline: sp(h+1) issued between
                    # exp(h) and pv(h); pair tails interleave two heads later
                    emit_sp(0)
                    for h in range(NH):
                        if h + 1 < NH:
                            emit_sp(h + 1)
                        emit_pv(h)
                        if h >= 2 and h % 2 == 1:
                            emit_norm(h // 2 - 1)
                            yield
                    emit_norm(3)
                    yield

                osb = work.tile([128, 2, C1], BF16, tag="osb")
                for tc in range(2):
                    fp = pp.tile([128, 2, NKV], F32, tag="ppB", name="fp", bufs=2)
                    kts = [0, 1, 2, 3]
                    for fh in range(2):
                        for kt in kts:
                            nc.tensor.matmul(
                                fp[:, fh],
                                outT[:, kt, tc * 128 : (tc + 1) * 128],
                                proj_s[:, kt, fh * 256 : (fh + 1) * 256],
                                start=(kt == 0),
                                stop=(kt == 3),
                                skip_group_check=True,
                            )
                    # alternate engines so the two bias-adds overlap, and
                    # store each half as soon as it is ready
                    nc.vector.tensor_add(
                        osb[:, tc],
                        fp.rearrange("p a b -> p (a b)"),
                        projb_s,
                    )
                    nc.sync.dma_start(out=outr[bi][:, tc], in_=osb[:, tc])
                    yield

            def _drain(g):
                if g is None:
                    return False
                try:
                    next(g)
                    return True
                except StopIteration:
                    return False

            # ---- software pipeline ------------------------------------
            # Window t round-robins chunks of S3(t-2) / S2ab(t-1) / S1y(t) /
            # S1x(t+1); window 0 also streams the big weights between band
            # DMAs and runs S1x(0).
            NB = reps * BPC
            states = {}
            for t in range(NB + 2):
                gens = []
                if t < NB:
                    states[t] = {}
                if t >= 2:
                    gens.append(s3_gen((t - 2) % BPC, states[t - 2]))
                if 1 <= t and t - 1 < NB:
                    gens.append(s2ab_gen((t - 1) % BPC, states[t - 1]))
                if t < NB:
                    gens.append(s1_gen(t % BPC, states[t],
                                       pre=pre0 if t == 0 else None))
                if t == 0:
                    gens.append(bigw_gen())
                while gens:
                    nxt = []
                    for g in gens:
                        try:
                            next(g)
                            nxt.append(g)
                        except StopIteration:
                            pass
                    gens = nxt
                if t >= 2:
                    del states[t - 2]

    _split_excess_waits(nc)
    return nc


def _split_fp8(w16):
    """w16 (f32) -> stacked [2, ...] fp8 hi/lo with hi+lo ~= w16."""
    f8 = ml_dtypes.float8_e4m3
    hi = w16.astype(f8)
    lo = (w16 - hi.astype(np.float32)).astype(f8)
    return np.stack([hi, lo], axis=0)


def _prep_common(inputs):
    Wq = np.asarray(inputs["Wq"], dtype=np.float32)
    Wkv = np.asarray(inputs["Wkv"], dtype=np.float32)
    proj_w = np.asarray(inputs["proj_w"], dtype=np.float32)
    proj_b = np.asarray(inputs["proj_b"], dtype=np.float32)

    bf = ml_dtypes.bfloat16
    f8 = ml_dtypes.float8_e4m3
    common = {
        # x16 prescale keeps the fp8 split residual out of subnormals; the
        # 16*16=256 score scale folds into the Exp activation, the 16x on v
        # folds into proj_t below.
        "wq_t": _split_fp8(np.ascontiguousarray(Wq.T) * 16.0),
        "wkv_t": _split_fp8(np.ascontiguousarray(Wkv.T[_PERM, :]) * 16.0),
        "proj_t": (np.ascontiguousarray(proj_w.T) / 16.0).astype(bf),
        "projb": proj_b.astype(bf),
    }
    ah = _pool_mats()
    for i in range(3):
        common[f"ah{i+1}"] = np.stack([ah[i], ah[i]], axis=1).astype(f8)
        c = C2[i]
        cpad = ((c + 127) // 128) * 128
        pr = min(c, 128)
        nkt = (c + 127) // 128
        srw_t = np.asarray(inputs[f"sr{i+1}_w"], dtype=np.float32).T  # [c_in, c_out]
        srw_p = np.zeros((nkt * pr, c), dtype=np.float32)
        srw_p[:c] = srw_t
        common[f"srw{i+1}_t"] = srw_p.astype(bf)
        common[f"srb{i+1}"] = np.asarray(
            inputs[f"sr{i+1}_b"], dtype=np.float32
        ).astype(bf)
    for i, c in enumerate((64, 128, 320, 512)):
        cpad = ((c + 127) // 128) * 128
        if i < 3:
            g = np.asarray(inputs[f"ln{i+1}_g"], dtype=np.float32)
            b = np.asarray(inputs[f"ln{i+1}_b"], dtype=np.float32)
        else:
            g = np.asarray(inputs["ln4_g"], dtype=np.float32)
            b = np.asarray(inputs["ln4_b"], dtype=np.float32)
        gp = np.zeros(cpad, dtype=np.float32)
        gp[:c] = g
        bp = np.zeros(cpad, dtype=np.float32)
        bp[:c] = b
        common[f"g{i+1}"] = gp
        common[f"lb{i+1}"] = bp
    return common


def kernel(**inputs):
    bf = ml_dtypes.bfloat16
    f8 = ml_dtypes.float8_e4m3
    x = np.ascontiguousarray(inputs["x"]).astype(bf)
    y1 = np.ascontiguousarray(inputs["y1"]).astype(f8)
    y2 = np.ascontiguousarray(inputs["y2"]).astype(f8)
    y3 = np.ascontiguousarray(inputs["y3"]).astype(f8)
    common = _prep_common(inputs)

    nc = build_module()
    in_maps = []
    for c in range(NCORES):
        sl = slice(c * BPC, (c + 1) * BPC)
        m = dict(common)
        m["x"] = x[sl]
        m["y1"] = y1[sl]
        m["y2"] = y2[sl]
        m["y3"] = y3[sl]
        in_maps.append(m)

    res = run_bass_kernel_spmd(nc, in_maps, core_ids=list(range(NCORES)))
    return np.concatenate(
        [np.asarray(r["out"]).astype(np.float32) for r in res.results], axis=0
    )


if __name__ == "__main__":
    pass

